# revision 1
# baseline (speedup 1.0000x reference)
"""Trainium2 Bass kernel for nn_DoubleLayeredEncoder (2-layer GCN, N=100k, E=1.6M).

Strategy (8 NeuronCores, SPMD, one NEFF):
  - Each core owns 6250 "lo" nodes [6250c, 6250(c+1)) and the paired 6250 "hi"
    nodes [50000+6250c, ...), so the final (n1+n2)/2 is core-local.
  - Edges are assigned to the core owning dst, sorted into 98 windows of 128
    dst slots, and within each window grouped by src chunk (4 chunks of the
    gather table, since dma_gather indices are int16).
  - Per 128-edge tile: one DVE tensor_scalar builds the one-hot selection
    matrix S[e,d] = (iota[d] == dst_slot[e]) * w[e]; the tensor engine
    accumulates psum[d,f] += S.T @ G where G = gathered source rows.
  - Source rows come from yw = dinv * (x @ W) tables: each core computes its
    shard (deg -> rsqrt -> scale), then an AllGather makes the full table
    available for dma_gather.  dinv[dst] is applied at window eviction.
  - Layer-2 dense matmul (h1 @ W2) is fused into layer-1 window eviction via
    a PE transpose.
"""

import math

import numpy as np


# ---------------------------------------------------------------------------
# Config
# ---------------------------------------------------------------------------
def make_cfg(n=100000, ncores=8, nchunk=4, wb=4):
    c = {}
    c["N"] = n
    c["IN_CH"] = 128
    c["C1"] = 128
    c["C2"] = 64
    c["NCORES"] = ncores
    c["HALF"] = n // 2
    c["PCH"] = c["HALF"] // ncores            # nodes per core per half
    c["OWN"] = 2 * c["PCH"]
    c["WPH"] = (c["PCH"] + 127) // 128        # windows per half
    c["NWIN"] = 2 * c["WPH"]
    c["SHARD_ROWS"] = c["NWIN"] * 128
    c["TABLE_ROWS"] = ncores * c["SHARD_ROWS"]
    c["NCHUNK"] = nchunk
    assert c["TABLE_ROWS"] % nchunk == 0
    c["CHUNK_ROWS"] = c["TABLE_ROWS"] // nchunk
    assert c["CHUNK_ROWS"] <= 32768, "dma_gather idx is int16"
    c["WB"] = wb
    return c


CFG = make_cfg()


def _row_of_node(c, j):
    """Row of node j in the allgathered (rank-block-concatenated) tables."""
    j = np.asarray(j)
    lo = j < c["HALF"]
    core = np.where(lo, j // c["PCH"], (j - c["HALF"]) // c["PCH"])
    pos = np.where(lo, j - core * c["PCH"], j - c["HALF"] - core * c["PCH"])
    return core * c["SHARD_ROWS"] + np.where(lo, pos, c["WPH"] * 128 + pos)


# ---------------------------------------------------------------------------
# Host-side prep: per-core edge tiles, metadata, gather indices
# ---------------------------------------------------------------------------
def prep(cfg, x, edge_index, edge_weight, edge_type):
    NCORES, NWIN, NCHUNK, WB = (cfg["NCORES"], cfg["NWIN"], cfg["NCHUNK"],
                                cfg["WB"])
    CHUNK_ROWS, SHARD_ROWS, PCH, HALF = (cfg["CHUNK_ROWS"], cfg["SHARD_ROWS"],
                                         cfg["PCH"], cfg["HALF"])
    src = np.asarray(edge_index[0], dtype=np.int64)
    dst = np.asarray(edge_index[1], dtype=np.int64)
    w = np.asarray(edge_weight, dtype=np.float32)
    t = np.asarray(edge_type, dtype=np.float32)

    src_row = _row_of_node(cfg, src).astype(np.int32)
    dst_row = _row_of_node(cfg, dst).astype(np.int32)
    core_of_edge = dst_row // SHARD_ROWS

    cores = []
    for c in range(NCORES):
        sel = core_of_edge == c
        e_src = src_row[sel]
        e_dstloc = dst_row[sel] - c * SHARD_ROWS
        e_w = w[sel]
        e_t = t[sel]
        # self loops (weight 1 in both layers) for the real owned nodes
        own_lo = np.arange(c * PCH, (c + 1) * PCH)
        own = np.concatenate([own_lo, own_lo + HALF])
        sl_row = _row_of_node(cfg, own).astype(np.int32)
        e_src = np.concatenate([e_src, sl_row])
        e_dstloc = np.concatenate([e_dstloc, sl_row - c * SHARD_ROWS])
        e_w = np.concatenate([e_w, np.ones(cfg["OWN"], np.float32)])
        e_t = np.concatenate([e_t, np.ones(cfg["OWN"], np.float32)])

        win = e_dstloc >> 7
        slot = e_dstloc & 127
        chunk = e_src // CHUNK_ROWS
        order = np.lexsort((chunk, win))
        cores.append(dict(src=e_src[order], slot=slot[order], w=e_w[order],
                          t=e_t[order], win=win[order], chunk=chunk[order]))

    counts = np.zeros((NCORES, NWIN, NCHUNK), np.int64)
    for c in range(NCORES):
        d = cores[c]
        np.add.at(counts[c], (d["win"], d["chunk"]), 1)
    tiles_wc = ((counts.max(axis=0) + 127) // 128).astype(np.int64)

    ntiles = int(tiles_wc.sum())
    slots = ntiles * 128

    nbatch = (NWIN + WB - 1) // WB
    calls = []
    for b in range(nbatch):
        wlo, whi = b * WB, min((b + 1) * WB, NWIN)
        for ch in range(NCHUNK):
            calls.append((b, ch, int(tiles_wc[wlo:whi, ch].sum())))
    max_call_tiles = max(cl[2] for cl in calls)

    wc_start = np.zeros((NWIN, NCHUNK), np.int64)
    acc = 0
    for wdx in range(NWIN):
        for ch in range(NCHUNK):
            wc_start[wdx, ch] = acc
            acc += int(tiles_wc[wdx, ch])

    per_core = []
    for c in range(NCORES):
        d = cores[c]
        meta = np.zeros((slots, 4), np.float32)   # dst_slot, w, t, pad
        gidx = np.zeros(slots, np.int32)
        pos = 0
        key = d["win"] * NCHUNK + d["chunk"]
        bounds = np.searchsorted(key, np.arange(NWIN * NCHUNK + 1))
        for wdx in range(NWIN):
            for ch in range(NCHUNK):
                k = wdx * NCHUNK + ch
                s, e = bounds[k], bounds[k + 1]
                n = e - s
                T = int(tiles_wc[wdx, ch])
                assert n <= T * 128
                meta[pos:pos + n, 0] = d["slot"][s:e]
                meta[pos:pos + n, 1] = d["w"][s:e]
                meta[pos:pos + n, 2] = d["t"][s:e]
                gidx[pos:pos + n] = d["src"][s:e]
                gidx[pos + n:pos + T * 128] = ch * CHUNK_ROWS  # valid pad row
                pos += T * 128
        assert pos == slots

        meta_blocks = np.zeros((len(calls), 128, max_call_tiles * 4),
                               np.float32)
        idx_blocks = np.full((len(calls), 128, max_call_tiles * 8),
                             -1, np.int16)
        for ci, (b, ch, tc) in enumerate(calls):
            if tc == 0:
                continue
            wlo, whi = b * WB, min((b + 1) * WB, NWIN)
            blk = np.concatenate(
                [meta[wc_start[wdx, ch] * 128:
                      (wc_start[wdx, ch] + int(tiles_wc[wdx, ch])) * 128]
                 for wdx in range(wlo, whi)], axis=0)
            gi = np.concatenate(
                [gidx[wc_start[wdx, ch] * 128:
                      (wc_start[wdx, ch] + int(tiles_wc[wdx, ch])) * 128]
                 for wdx in range(wlo, whi)], axis=0) - ch * CHUNK_ROWS
            assert blk.shape[0] == tc * 128
            m = blk.reshape(tc, 128, 4).transpose(1, 0, 2).reshape(128, tc * 4)
            meta_blocks[ci, :, :tc * 4] = m
            assert gi.min() >= 0 and gi.max() < CHUNK_ROWS
            # dma_gather idx layout: idx j at [partition j%16, column j//16],
            # replicated across the 8 Q7 core groups
            cols = tc * 128 // 16
            lay = gi.astype(np.int16).reshape(cols, 16).T
            idx_blocks[ci, :, :cols] = np.tile(lay, (8, 1))

        xsh = np.zeros((SHARD_ROWS, cfg["IN_CH"]), np.float32)
        own_lo = np.arange(c * PCH, (c + 1) * PCH)
        xsh[:PCH] = x[own_lo]
        xsh[cfg["WPH"] * 128:cfg["WPH"] * 128 + PCH] = x[own_lo + HALF]
        xT = np.ascontiguousarray(xsh.T)

        per_core.append(dict(meta=meta_blocks, idx=idx_blocks, xT=xT))

    structure = dict(tiles_wc=tiles_wc, calls=calls, ntiles=ntiles,
                     max_call_tiles=max_call_tiles, nbatch=nbatch)
    return structure, per_core


def _tile_iter(cfg, structure):
    """Yields (call_index, tile_within_call) in (window, chunk, tile) order."""
    tiles_wc = structure["tiles_wc"]
    calls = structure["calls"]
    call_idx = {(b, ch): i for i, (b, ch, _) in enumerate(calls)}
    cursor = [0] * len(calls)
    for wdx in range(cfg["NWIN"]):
        b = wdx // cfg["WB"]
        for ch in range(cfg["NCHUNK"]):
            ci = call_idx[(b, ch)]
            for _ in range(int(structure["tiles_wc"][wdx, ch])):
                yield ci, cursor[ci]
                cursor[ci] += 1


# ---------------------------------------------------------------------------
# Numpy emulation of the exact device algorithm (debug/validation)
# ---------------------------------------------------------------------------
def emulate(cfg, structure, per_core, W1, b1, a1, W2, b2, a2):
    NWIN, NCHUNK, NCORES = cfg["NWIN"], cfg["NCHUNK"], cfg["NCORES"]
    WPH, PCH, C1, C2 = cfg["WPH"], cfg["PCH"], cfg["C1"], cfg["C2"]
    tiles_wc = structure["tiles_wc"]
    calls = structure["calls"]
    iota = np.arange(128, dtype=np.float32)

    yw1_shards, dinv_all = [], []
    for c in range(NCORES):
        meta = per_core[c]["meta"]
        dinv = np.zeros((NWIN, 128, 2), np.float32)
        ti = _tile_iter(cfg, structure)
        for wdx in range(NWIN):
            deg = np.zeros((128, 2), np.float32)
            for ch in range(NCHUNK):
                for _ in range(int(tiles_wc[wdx, ch])):
                    ci, tloc = next(ti)
                    m = meta[ci][:, tloc * 4:tloc * 4 + 4]
                    onehot = (iota[None, :] == m[:, 0:1])
                    deg += onehot.T.astype(np.float32) @ m[:, 1:3]
            dinv[wdx] = 1.0 / np.sqrt(np.maximum(deg, 1e-12))
        dinv_all.append(dinv)
        xT = per_core[c]["xT"]
        yw1_shards.append((xT.T @ W1) * dinv[:, :, 0].reshape(-1, 1))
    yw1_full = np.concatenate(yw1_shards, 0)

    yw2_shards = []
    for c in range(NCORES):
        meta, idxb = per_core[c]["meta"], per_core[c]["idx"]
        dinv = dinv_all[c]
        yw2 = np.zeros((cfg["SHARD_ROWS"], C2), np.float32)
        ti = _tile_iter(cfg, structure)
        gathered = _emu_gather(cfg, idxb, calls, yw1_full, C1)
        for wdx in range(NWIN):
            acc = np.zeros((128, C1), np.float32)
            for ch in range(NCHUNK):
                for _ in range(int(tiles_wc[wdx, ch])):
                    ci, tloc = next(ti)
                    m = meta[ci][:, tloc * 4:tloc * 4 + 4]
                    S = (iota[None, :] == m[:, 0:1]) * m[:, 1:2]
                    G = gathered[ci][:, tloc * C1:(tloc + 1) * C1]
                    acc += S.T @ G
            z = acc * dinv[wdx, :, 0:1] + b1[None, :]
            h1 = np.maximum(z, 0) + a1[None, :] * np.minimum(z, 0)
            yw2[wdx * 128:(wdx + 1) * 128] = (h1 @ W2) * dinv[wdx, :, 1:2]
        yw2_shards.append(yw2)
    yw2_full = np.concatenate(yw2_shards, 0)

    outs = []
    for c in range(NCORES):
        meta, idxb = per_core[c]["meta"], per_core[c]["idx"]
        dinv = dinv_all[c]
        ti = _tile_iter(cfg, structure)
        gathered = _emu_gather(cfg, idxb, calls, yw2_full, C2)
        h2 = np.zeros((NWIN, 128, C2), np.float32)
        for wdx in range(NWIN):
            acc = np.zeros((128, C2), np.float32)
            for ch in range(NCHUNK):
                for _ in range(int(tiles_wc[wdx, ch])):
                    ci, tloc = next(ti)
                    m = meta[ci][:, tloc * 4:tloc * 4 + 4]
                    S = (iota[None, :] == m[:, 0:1]) * m[:, 2:3]
                    G = gathered[ci][:, tloc * C2:(tloc + 1) * C2]
                    acc += S.T @ G
            z = acc * dinv[wdx, :, 1:2] + b2[None, :]
            h2[wdx] = np.maximum(z, 0) + a2[None, :] * np.minimum(z, 0)
        lo = h2[:WPH].reshape(-1, C2)[:PCH]
        hi = h2[WPH:].reshape(-1, C2)[:PCH]
        outs.append((lo + hi) * 0.5)
    return np.concatenate(outs, 0)


def _emu_gather(cfg, idx_blocks, calls, table, width):
    out = []
    for ci, (b, ch, tc) in enumerate(calls):
        g = np.zeros((128, max(tc, 1) * width), np.float32)
        if tc:
            cols = tc * 128 // 16
            lay = idx_blocks[ci][:16, :cols]
            idx = lay.T.reshape(-1).astype(np.int64) + ch * cfg["CHUNK_ROWS"]
            rows = table[idx]
            g = rows.reshape(tc, 128, width).transpose(1, 2, 0).transpose(
                0, 2, 1).reshape(128, tc * width)
        out.append(g)
    return out


# ---------------------------------------------------------------------------
# Bass kernel builder
# ---------------------------------------------------------------------------
def build_bass(cfg, structure):
    import os

    import concourse.bass as bass
    import concourse.tile as tile
    from concourse import bacc as bacc_mod
    from concourse import mybir

    stop = os.environ.get("GCN_STOP", "full")  # A | B | C | full

    NWIN, NCHUNK, WB, WPH = cfg["NWIN"], cfg["NCHUNK"], cfg["WB"], cfg["WPH"]
    C1, C2 = cfg["C1"], cfg["C2"]
    SHARD_ROWS, TABLE_ROWS, CHUNK_ROWS = (cfg["SHARD_ROWS"],
                                          cfg["TABLE_ROWS"],
                                          cfg["CHUNK_ROWS"])
    tiles_wc = structure["tiles_wc"]
    calls = structure["calls"]
    mct = structure["max_call_tiles"]
    ncalls = len(calls)
    f32 = mybir.dt.float32
    AF = mybir.ActivationFunctionType
    OP = mybir.AluOpType

    # Bacc (not plain Bass): finalize() runs the TRN2 legalization passes
    # (sync-wait splitting, custom-ISA codegen, library load insertion).
    nc = bacc_mod.Bacc(num_devices=cfg["NCORES"])

    # I/O
    meta_d = nc.declare_dram_parameter("meta", [ncalls * 128, mct * 4], f32,
                                       isOutput=False)
    idx_d = nc.declare_dram_parameter("idx", [ncalls * 128, mct * 8],
                                      mybir.dt.int16, isOutput=False)
    xT_d = nc.declare_dram_parameter("xT", [128, SHARD_ROWS], f32,
                                     isOutput=False)
    W1_d = nc.declare_dram_parameter("W1", [128, C1], f32, isOutput=False)
    W2_d = nc.declare_dram_parameter("W2", [C1, C2], f32, isOutput=False)
    b1_d = nc.declare_dram_parameter("b1r", [128, C1], f32, isOutput=False)
    a1_d = nc.declare_dram_parameter("a1r", [128, C1], f32, isOutput=False)
    b2_d = nc.declare_dram_parameter("b2r", [128, C2], f32, isOutput=False)
    a2_d = nc.declare_dram_parameter("a2r", [128, C2], f32, isOutput=False)
    iota_d = nc.declare_dram_parameter("iota", [128, 128], f32, isOutput=False)
    ident_d = nc.declare_dram_parameter("ident", [128, 128], f32,
                                        isOutput=False)
    out_d = nc.declare_dram_parameter("out", [WPH * 128, C2], f32,
                                      isOutput=True)

    rg = [list(range(cfg["NCORES"]))]

    with tile.TileContext(nc, num_cores=cfg["NCORES"]) as tc_:
        with (
            tc_.tile_pool(name="const", bufs=1) as constp,
            tc_.tile_pool(name="dinv", bufs=1) as dinvp,
            tc_.tile_pool(name="meta", bufs=6) as metap,
            tc_.tile_pool(name="idx", bufs=6) as idxp,
            tc_.tile_pool(name="g", bufs=6) as gp,
            tc_.tile_pool(name="s", bufs=4) as sp,
            tc_.tile_pool(name="ev", bufs=3) as evp,
            tc_.tile_pool(name="stash", bufs=1) as stashp,
            tc_.tile_pool(name="xtp", bufs=3) as xtp,
            tc_.tile_pool(name="degps", bufs=2, space="PSUM") as degps,
            tc_.tile_pool(name="winps", bufs=2, space="PSUM") as winps,
            tc_.tile_pool(name="tps", bufs=2, space="PSUM") as tps,
            tc_.tile_pool(name="y2ps", bufs=2, space="PSUM") as y2ps,
            tc_.tile_pool(name="dram", bufs=1, space="DRAM") as dramp,
        ):
            # ---- constants into SBUF
            iota_sb = constp.tile([128, 128], f32, name="iota_sb")
            ident_sb = constp.tile([128, 128], f32, name="ident_sb")
            W1_sb = constp.tile([128, C1], f32, name="W1_sb")
            W2_sb = constp.tile([C1, C2], f32, name="W2_sb")
            b1_sb = constp.tile([128, C1], f32, name="b1_sb")
            a1_sb = constp.tile([128, C1], f32, name="a1_sb")
            b2_sb = constp.tile([128, C2], f32, name="b2_sb")
            a2_sb = constp.tile([128, C2], f32, name="a2_sb")
            for sb, dr in ((iota_sb, iota_d), (ident_sb, ident_d),
                           (W1_sb, W1_d), (W2_sb, W2_d), (b1_sb, b1_d),
                           (a1_sb, a1_d), (b2_sb, b2_d), (a2_sb, a2_d)):
                nc.sync.dma_start(out=sb, in_=dr[:, :])

            dinv_sb = dinvp.tile([128, NWIN * 2], f32, name="dinv_sb")

            # DRAM scratch
            yw1_shard = dramp.tile([SHARD_ROWS, C1], f32, name="yw1_shard")
            yw1_full = dramp.tile([TABLE_ROWS, C1], f32, name="yw1_full",
                                  addr_space="Shared")
            yw2_shard = dramp.tile([SHARD_ROWS, C2], f32, name="yw2_shard")
            yw2_full = dramp.tile([TABLE_ROWS, C2], f32, name="yw2_full",
                                  addr_space="Shared")

            call_of = {(b, ch): i for i, (b, ch, _) in enumerate(calls)}

            # one Pool register per distinct num_idxs value (to_reg allocates
            # a fresh register per call and the register file is small)
            _nreg_cache = {}

            def nreg(v):
                if v not in _nreg_cache:
                    _nreg_cache[v] = nc.gpsimd.to_reg(v)
                return _nreg_cache[v]

            # ================= pass A: degrees -> dinv =================
            meta_tiles = {}

            def load_meta(ci):
                t = metap.tile([128, mct * 4], f32, tag="meta")
                nc.sync.dma_start(out=t,
                                  in_=meta_d[ci * 128:(ci + 1) * 128, :])
                return t

            cursor = [0] * ncalls
            for b in range(structure["nbatch"]):
                for ch in range(NCHUNK):
                    ci = call_of[(b, ch)]
                    if calls[ci][2]:
                        meta_tiles[ci] = load_meta(ci)
                wlo = b * WB
                whi = min(wlo + WB, NWIN)
                for wdx in range(wlo, whi):
                    ntile_w = int(tiles_wc[wdx].sum())
                    deg_ps = degps.tile([128, 2], f32, tag="deg")
                    k = 0
                    for ch in range(NCHUNK):
                        ci = call_of[(b, ch)]
                        for _ in range(int(tiles_wc[wdx, ch])):
                            tloc = cursor[ci]
                            cursor[ci] += 1
                            m = meta_tiles[ci]
                            s_t = sp.tile([128, 128], f32, tag="s")
                            nc.vector.tensor_scalar(
                                out=s_t, in0=iota_sb,
                                scalar1=m[:, 4 * tloc:4 * tloc + 1],
                                scalar2=None, op0=OP.is_equal)
                            nc.tensor.matmul(
                                out=deg_ps, lhsT=s_t,
                                rhs=m[:, 4 * tloc + 1:4 * tloc + 3],
                                start=(k == 0), stop=(k == ntile_w - 1))
                            k += 1
                    # dinv = 1/sqrt(deg); deg >= 1 (self loop).  Rsqrt is
                    # banned in bass (accuracy) -> reciprocal then sqrt.
                    rec_t = evp.tile([128, 2], f32, tag="rec")
                    nc.vector.reciprocal(out=rec_t, in_=deg_ps)
                    nc.scalar.activation(
                        out=dinv_sb[:, 2 * wdx:2 * wdx + 2], in_=rec_t,
                        func=AF.Sqrt)

            # ================= pass B: yw1 shard + AllGather ============
            if stop == "A":
                nc.sync.dma_start(out=out_d[0:128, :],
                                  in_=dinv_sb[:, 0:C2])
            if stop in ("B", "C0", "C1", "C", "full"):
                for wdx in range(NWIN):
                    xt_t = xtp.tile([128, 128], f32, tag="xt")
                    nc.sync.dma_start(out=xt_t,
                                      in_=xT_d[:, wdx * 128:(wdx + 1) * 128])
                    xw_ps = y2ps.tile([128, C1], f32, tag="y2")
                    nc.tensor.matmul(out=xw_ps, lhsT=xt_t, rhs=W1_sb,
                                     start=True, stop=True)
                    yw_t = evp.tile([128, C1], f32, tag="yw")
                    nc.vector.tensor_scalar(
                        out=yw_t, in0=xw_ps,
                        scalar1=dinv_sb[:, 2 * wdx:2 * wdx + 1],
                        scalar2=None, op0=OP.mult)
                    nc.sync.dma_start(
                        out=yw1_shard[wdx * 128:(wdx + 1) * 128, :], in_=yw_t)

                nc.gpsimd.collective_compute(
                    "AllGather", OP.bypass, replica_groups=rg,
                    ins=[yw1_shard[:, :]], outs=[yw1_full[:, :]])
            if stop == "B":
                t_dbg = evp.tile([128, C2], f32, tag="dbg")
                nc.sync.dma_start(out=t_dbg, in_=yw1_full[0:128, 0:C2])
                nc.sync.dma_start(out=out_d[0:128, :], in_=t_dbg)

            # ============ pass C: layer-1 messages, fused yw2 ===========
            def msg_pass(table, width, wcol, dcol, b_sb, a_sb, out_cb):
                """wcol: meta column for edge weight (1=w, 2=type);
                dcol: dinv column (0 or 1) used at eviction;
                out_cb(wdx, h_tile): consume the [128, width] result."""
                cursor = [0] * ncalls
                for b in range(structure["nbatch"]):
                    g_tiles = {}
                    for ch in range(NCHUNK):
                        ci = call_of[(b, ch)]
                        tcn = calls[ci][2]
                        if not tcn:
                            continue
                        meta_tiles[ci] = load_meta(ci)
                        it = idxp.tile([128, mct * 8], mybir.dt.int16,
                                       tag="idx")
                        nc.sync.dma_start(
                            out=it, in_=idx_d[ci * 128:(ci + 1) * 128, :])
                        g_t = gp.tile([128, mct * C1], f32, tag="g")
                        if not os.environ.get("GCN_NOGATHER"):
                            nc.gpsimd.dma_gather(
                                out_ap=g_t[:, :tcn * width].rearrange(
                                    "p (t e) -> p t e", e=width),
                                in_ap=table[ch * CHUNK_ROWS:
                                            (ch + 1) * CHUNK_ROWS, :],
                                idxs_ap=it[:, :tcn * 8],
                                num_idxs=tcn * 128,
                                num_idxs_reg=nreg(tcn * 128),
                                elem_size=width,
                                # single_packet=True breaks for calls over
                                # ~384 indices (HW-bisected)
                                single_packet=False)
                        else:
                            nc.vector.tensor_copy(out=g_t[:, 0:128],
                                                  in_=iota_sb)
                        g_tiles[ch] = g_t
                    wlo = b * WB
                    whi = min(wlo + WB, NWIN)
                    for wdx in range(wlo, whi):
                        ntile_w = int(tiles_wc[wdx].sum())
                        h_ps = winps.tile([128, width], f32, tag="win")
                        k = 0
                        for ch in range(NCHUNK):
                            ci = call_of[(b, ch)]
                            for _ in range(int(tiles_wc[wdx, ch])):
                                tloc = cursor[ci]
                                cursor[ci] += 1
                                m = meta_tiles[ci]
                                s_t = sp.tile([128, 128], f32, tag="s")
                                nc.vector.tensor_scalar(
                                    out=s_t, in0=iota_sb,
                                    scalar1=m[:, 4 * tloc:4 * tloc + 1],
                                    scalar2=m[:, 4 * tloc + wcol:
                                              4 * tloc + wcol + 1],
                                    op0=OP.is_equal, op1=OP.mult)
                                nc.tensor.matmul(
                                    out=h_ps, lhsT=s_t,
                                    rhs=g_tiles[ch][:, tloc * width:
                                                    (tloc + 1) * width],
                                    start=(k == 0), stop=(k == ntile_w - 1))
                                k += 1
                        # evict: z = psum * dinv + b ; h = prelu(z, a)
                        if os.environ.get("GCN_NOEVICT"):
                            h_t = evp.tile([128, width], f32, tag="h")
                            nc.vector.tensor_copy(out=h_t, in_=h_ps)
                            out_cb(wdx, h_t)
                            continue
                        dv = dinv_sb[:, 2 * wdx + dcol:2 * wdx + dcol + 1]
                        z_t = evp.tile([128, width], f32, tag="z")
                        nc.vector.scalar_tensor_tensor(
                            out=z_t, in0=h_ps, scalar=dv, in1=b_sb,
                            op0=OP.mult, op1=OP.add)
                        mn_t = evp.tile([128, width], f32, tag="mn")
                        nc.vector.tensor_scalar(
                            out=mn_t, in0=z_t, scalar1=0.0, scalar2=None,
                            op0=OP.min)
                        am_t = evp.tile([128, width], f32, tag="am")
                        nc.vector.tensor_tensor(out=am_t, in0=mn_t, in1=a_sb,
                                                op=OP.mult)
                        h_t = evp.tile([128, width], f32, tag="h")
                        nc.vector.scalar_tensor_tensor(
                            out=h_t, in0=z_t, scalar=0.0, in1=am_t,
                            op0=OP.max, op1=OP.add)
                        out_cb(wdx, h_t)

            def l1_out(wdx, h_t):
                # fused layer-2 dense: yw2 = (h1 @ W2) * dinv2
                t_ps = tps.tile([128, 128], f32, tag="tp")
                nc.tensor.transpose(out=t_ps, in_=h_t, identity=ident_sb)
                h1T = evp.tile([128, 128], f32, tag="h1T")
                nc.vector.tensor_copy(out=h1T, in_=t_ps)
                y2_ps = y2ps.tile([128, C2], f32, tag="y2")
                nc.tensor.matmul(out=y2_ps, lhsT=h1T, rhs=W2_sb,
                                 start=True, stop=True)
                yw2_t = evp.tile([128, C2], f32, tag="yw2")
                nc.vector.tensor_scalar(
                    out=yw2_t, in0=y2_ps,
                    scalar1=dinv_sb[:, 2 * wdx + 1:2 * wdx + 2],
                    scalar2=None, op0=OP.mult)
                nc.sync.dma_start(
                    out=yw2_shard[wdx * 128:(wdx + 1) * 128, :], in_=yw2_t)

            def l1_out_nofuse(wdx, h_t):
                nc.sync.dma_start(
                    out=yw2_shard[wdx * 128:(wdx + 1) * 128, :],
                    in_=h_t[:, :C2])

            if stop in ("C0",):
                msg_pass(yw1_full, C1, 1, 0, b1_sb, a1_sb, l1_out_nofuse)
            if stop in ("C1",):
                msg_pass(yw1_full, C1, 1, 0, b1_sb, a1_sb, l1_out)
            if stop in ("C", "full"):
                msg_pass(yw1_full, C1, 1, 0, b1_sb, a1_sb, l1_out)

                nc.gpsimd.collective_compute(
                    "AllGather", OP.bypass, replica_groups=rg,
                    ins=[yw2_shard[:, :]], outs=[yw2_full[:, :]])
            if stop == "C":
                t_dbg = evp.tile([128, C2], f32, tag="dbg")
                nc.sync.dma_start(out=t_dbg, in_=yw2_full[0:128, :])
                nc.sync.dma_start(out=out_d[0:128, :], in_=t_dbg)
            if stop in ("C0", "C1"):
                t_dbg = evp.tile([128, C2], f32, tag="dbg")
                nc.sync.dma_start(out=t_dbg, in_=yw2_shard[0:128, :])
                nc.sync.dma_start(out=out_d[0:128, :], in_=t_dbg)

            # ============ pass E: layer-2 messages + combine ============
            stash = stashp.tile([128, WPH * C2], f32, name="h2lo")

            def l2_out(wdx, h_t):
                if wdx < WPH:
                    nc.vector.tensor_copy(
                        out=stash[:, wdx * C2:(wdx + 1) * C2], in_=h_t)
                else:
                    w2 = wdx - WPH
                    cmb = evp.tile([128, C2], f32, tag="cmb")
                    nc.vector.tensor_tensor(
                        out=cmb, in0=h_t,
                        in1=stash[:, w2 * C2:(w2 + 1) * C2], op=OP.add)
                    o_t = evp.tile([128, C2], f32, tag="o")
                    nc.vector.tensor_scalar(
                        out=o_t, in0=cmb, scalar1=0.5, scalar2=None,
                        op0=OP.mult)
                    nc.sync.dma_start(
                        out=out_d[w2 * 128:(w2 + 1) * 128, :], in_=o_t)

            if stop == "full":
                msg_pass(yw2_full, C2, 2, 1, b2_sb, a2_sb, l2_out)

    nc.finalize()
    return nc


# ---------------------------------------------------------------------------
# Host driver
# ---------------------------------------------------------------------------
def make_in_maps(cfg, structure, per_core, W1, b1, a1, W2, b2, a2):
    mct = structure["max_call_tiles"]
    ncalls = len(structure["calls"])
    iota = np.tile(np.arange(128, dtype=np.float32), (128, 1))
    ident = np.eye(128, dtype=np.float32)
    consts = dict(
        W1=np.ascontiguousarray(W1, np.float32),
        W2=np.ascontiguousarray(W2, np.float32),
        b1r=np.tile(b1.astype(np.float32), (128, 1)),
        a1r=np.tile(a1.astype(np.float32), (128, 1)),
        b2r=np.tile(b2.astype(np.float32), (128, 1)),
        a2r=np.tile(a2.astype(np.float32), (128, 1)),
        iota=np.ascontiguousarray(iota),
        ident=ident,
    )
    in_maps = []
    for c in range(cfg["NCORES"]):
        pc = per_core[c]
        in_maps.append(dict(
            meta=pc["meta"].reshape(ncalls * 128, mct * 4),
            idx=pc["idx"].reshape(ncalls * 128, mct * 8),
            xT=pc["xT"],
            **consts,
        ))
    return in_maps


def assemble_out(cfg, outs):
    """outs: list per core of the 'out' array [WPH*128, C2]."""
    parts = [o[:cfg["PCH"]] for o in outs]
    return np.ascontiguousarray(np.concatenate(parts, 0), dtype=np.float32)


LAST_EXEC_NS = None


def _trivial_nc(ncores):
    """A minimal bass kernel for dispatch-overhead calibration."""
    from concourse import bacc as bacc_mod
    from concourse import mybir
    import concourse.tile as tile

    f32 = mybir.dt.float32
    nc = bacc_mod.Bacc(num_devices=ncores)
    a = nc.declare_dram_parameter("a", [128, 128], f32, isOutput=False)
    o = nc.declare_dram_parameter("o", [128, 128], f32, isOutput=True)
    with tile.TileContext(nc, num_cores=ncores) as tc:
        with tc.tile_pool(name="p", bufs=2) as p:
            t = p.tile([128, 128], f32)
            nc.sync.dma_start(out=t, in_=a[:, :])
            nc.sync.dma_start(out=o[:, :], in_=t)
    nc.finalize()
    return nc


def _time_kernel(nc, in_maps, n_cores, iters=5, reps=10):
    """Best-of-N wall time of one dispatched execution (device put inputs,
    block_until_ready).  Subtract a trivial-kernel baseline for HW time."""
    import time

    import jax
    import numpy as np
    from jax.experimental.shard_map import shard_map
    from jax.sharding import Mesh, PartitionSpec

    from concourse import bass2jax, mybir

    bass2jax.install_neuronx_cc_hook()
    partition_name = (nc.partition_id_tensor.name
                      if nc.partition_id_tensor else None)
    in_names, out_names, out_avals, zero_outs = [], [], [], []
    for alloc in nc.m.functions[0].allocations:
        if not isinstance(alloc, mybir.MemoryLocationSet):
            continue
        name = alloc.memorylocations[0].name
        if alloc.kind == "ExternalInput":
            if name != partition_name:
                in_names.append(name)
        elif alloc.kind == "ExternalOutput":
            out_names.append(name)
            shape = tuple(alloc.tensor_shape)
            dtype = mybir.dt.np(alloc.dtype)
            out_avals.append(jax.core.ShapedArray(shape, dtype))
            zero_outs.append(np.zeros(shape, dtype))
    n_params = len(in_names)
    all_in_names = list(in_names) + list(out_names)
    if partition_name is not None:
        all_in_names.append(partition_name)

    n_outs_ = len(out_names)

    def make_body(n):
        # each iteration gets its own zero-buffer parameters: operands stay
        # top-level parameters (hook requirement) and differ across
        # iterations (no CSE); the effectful primitive keeps them ordered.
        def _body(*args):
            ins = list(args[:n_params])
            outs = None
            for i in range(n):
                zs = list(args[n_params + i * n_outs_:
                               n_params + (i + 1) * n_outs_])
                operands = ins + zs
                if partition_name is not None:
                    operands.append(bass2jax.partition_id_tensor())
                outs = bass2jax._bass_exec_p.bind(
                    *operands,
                    out_avals=tuple(out_avals),
                    in_names=tuple(all_in_names),
                    out_names=tuple(out_names),
                    lowering_input_output_aliases=(),
                    sim_require_finite=False,
                    sim_require_nnan=False,
                    nc=nc,
                )
            return tuple(outs)
        return _body

    devices = jax.devices()[:n_cores]
    mesh = Mesh(np.asarray(devices), ("core",))

    per_core = [[np.asarray(m[name]) for name in in_names] for m in in_maps]
    concat_in = [np.concatenate([per_core[c][i] for c in range(n_cores)], 0)
                 for i in range(n_params)]
    concat_zeros = [np.zeros((n_cores * z.shape[0], *z.shape[1:]), z.dtype)
                    for z in zero_outs]
    dev_in = [jax.device_put(a) for a in concat_in]
    dev_zero = [jax.device_put(a) for a in concat_zeros]

    n = 1
    in_specs = (PartitionSpec("core"),) * (n_params + n * n_outs_)
    out_specs = (PartitionSpec("core"),) * n_outs_
    fn = jax.jit(shard_map(make_body(n), mesh=mesh, in_specs=in_specs,
                           out_specs=out_specs, check_rep=False),
                 keep_unused=True)
    args = dev_in + dev_zero * n
    out = fn(*args)  # compile + warm
    jax.block_until_ready(out)
    times = []
    for _ in range(reps):
        t0 = time.perf_counter()
        out = fn(*args)
        jax.block_until_ready(out)
        times.append(time.perf_counter() - t0)
    times.sort()
    print(f"  timing: wall times ms = "
          f"{[f'{t*1e3:.2f}' for t in times[:8]]}")
    return times[0] * 1e9


def kernel(**inputs):
    global LAST_EXEC_NS
    import os

    cfg = CFG
    x = np.asarray(inputs["x"], np.float32)
    W1 = np.asarray(inputs["W1"], np.float32)
    b1 = np.asarray(inputs["b1"], np.float32)
    a1 = np.asarray(inputs["a1"], np.float32)
    W2 = np.asarray(inputs["W2"], np.float32)
    b2 = np.asarray(inputs["b2"], np.float32)
    a2 = np.asarray(inputs["a2"], np.float32)

    structure, per_core = prep(cfg, x, inputs["edge_index"],
                               inputs["edge_weight"], inputs["edge_type"])

    if os.environ.get("GCN_EMULATE"):
        return emulate(cfg, structure, per_core, W1, b1, a1, W2, b2, a2)

    from concourse.bass_utils import run_bass_kernel_spmd

    nc = build_bass(cfg, structure)
    in_maps = make_in_maps(cfg, structure, per_core, W1, b1, a1, W2, b2, a2)
    res = run_bass_kernel_spmd(
        nc, in_maps, core_ids=list(range(cfg["NCORES"])))
    LAST_EXEC_NS = res.exec_time_ns
    if os.environ.get("GCN_TIME"):
        main_ns = _time_kernel(nc, in_maps, cfg["NCORES"])
        triv = _trivial_nc(cfg["NCORES"])
        triv_ins = [dict(a=np.zeros((128, 128), np.float32))
                    for _ in range(cfg["NCORES"])]
        base_ns = _time_kernel(triv, triv_ins, cfg["NCORES"])
        print(f"  main {main_ns/1e6:.3f} ms, baseline {base_ns/1e6:.3f} ms")
        LAST_EXEC_NS = main_ns - base_ns
    return assemble_out(cfg, [res.results[c]["out"]
                              for c in range(cfg["NCORES"])])



# revision 10
# speedup vs baseline: 1.1304x; 1.1304x over previous
"""Trainium2 Bass kernel for nn_DoubleLayeredEncoder (2-layer GCN, N=100k, E=1.6M).

Strategy (8 NeuronCores, SPMD, one NEFF):
  - Each core owns 6250 "lo" nodes [6250c, 6250(c+1)) and the paired 6250 "hi"
    nodes [50000+6250c, ...), so the final (n1+n2)/2 is core-local.
  - Edges are assigned to the core owning dst, sorted into 98 windows of 128
    dst slots, and within each window grouped by src chunk (4 chunks of the
    gather table, since dma_gather indices are int16).
  - Per 128-edge tile: one DVE tensor_scalar builds the one-hot selection
    matrix S[e,d] = (iota[d] == dst_slot[e]) * w[e]; the tensor engine
    accumulates psum[d,f] += S.T @ G where G = gathered source rows.
  - Source rows come from yw = dinv * (x @ W) tables: each core computes its
    shard (deg -> rsqrt -> scale), then an AllGather makes the full table
    available for dma_gather.  dinv[dst] is applied at window eviction.
  - Layer-2 dense matmul (h1 @ W2) is fused into layer-1 window eviction via
    a PE transpose.
"""

import math

import numpy as np


# ---------------------------------------------------------------------------
# Config
# ---------------------------------------------------------------------------
def make_cfg(n=100000, ncores=8, nchunk=4, wb=4):
    c = {}
    c["N"] = n
    c["IN_CH"] = 128
    c["C1"] = 128
    c["C2"] = 64
    c["NCORES"] = ncores
    c["HALF"] = n // 2
    c["PCH"] = c["HALF"] // ncores            # nodes per core per half
    c["OWN"] = 2 * c["PCH"]
    c["WPH"] = (c["PCH"] + 127) // 128        # windows per half
    c["NWIN"] = 2 * c["WPH"]
    c["SHARD_ROWS"] = c["NWIN"] * 128
    c["TABLE_ROWS"] = ncores * c["SHARD_ROWS"]
    c["NCHUNK"] = nchunk
    assert c["TABLE_ROWS"] % nchunk == 0
    c["CHUNK_ROWS"] = c["TABLE_ROWS"] // nchunk
    assert c["CHUNK_ROWS"] <= 32768, "dma_gather idx is int16"
    c["WB"] = wb
    return c


CFG = make_cfg()


def _row_of_node(c, j):
    """Row of node j in the allgathered (rank-block-concatenated) tables."""
    j = np.asarray(j)
    lo = j < c["HALF"]
    core = np.where(lo, j // c["PCH"], (j - c["HALF"]) // c["PCH"])
    pos = np.where(lo, j - core * c["PCH"], j - c["HALF"] - core * c["PCH"])
    return core * c["SHARD_ROWS"] + np.where(lo, pos, c["WPH"] * 128 + pos)


# ---------------------------------------------------------------------------
# Host-side prep: per-core edge tiles, metadata, gather indices
# ---------------------------------------------------------------------------
def prep(cfg, x, edge_index, edge_weight, edge_type):
    NCORES, NWIN, NCHUNK, WB = (cfg["NCORES"], cfg["NWIN"], cfg["NCHUNK"],
                                cfg["WB"])
    CHUNK_ROWS, SHARD_ROWS, PCH, HALF = (cfg["CHUNK_ROWS"], cfg["SHARD_ROWS"],
                                         cfg["PCH"], cfg["HALF"])
    src = np.asarray(edge_index[0], dtype=np.int64)
    dst = np.asarray(edge_index[1], dtype=np.int64)
    w = np.asarray(edge_weight, dtype=np.float32)
    t = np.asarray(edge_type, dtype=np.float32)

    src_row = _row_of_node(cfg, src).astype(np.int32)
    dst_row = _row_of_node(cfg, dst).astype(np.int32)
    core_of_edge = dst_row // SHARD_ROWS

    cores = []
    for c in range(NCORES):
        sel = core_of_edge == c
        e_src = src_row[sel]
        e_dstloc = dst_row[sel] - c * SHARD_ROWS
        e_w = w[sel]
        e_t = t[sel]
        # self loops (weight 1 in both layers) for the real owned nodes
        own_lo = np.arange(c * PCH, (c + 1) * PCH)
        own = np.concatenate([own_lo, own_lo + HALF])
        sl_row = _row_of_node(cfg, own).astype(np.int32)
        e_src = np.concatenate([e_src, sl_row])
        e_dstloc = np.concatenate([e_dstloc, sl_row - c * SHARD_ROWS])
        e_w = np.concatenate([e_w, np.ones(cfg["OWN"], np.float32)])
        e_t = np.concatenate([e_t, np.ones(cfg["OWN"], np.float32)])

        win = e_dstloc >> 7
        slot = e_dstloc & 127
        chunk = e_src // CHUNK_ROWS
        order = np.lexsort((chunk, win))
        cores.append(dict(src=e_src[order], slot=slot[order], w=e_w[order],
                          t=e_t[order], win=win[order], chunk=chunk[order]))

    counts = np.zeros((NCORES, NWIN, NCHUNK), np.int64)
    for c in range(NCORES):
        d = cores[c]
        np.add.at(counts[c], (d["win"], d["chunk"]), 1)
    tiles_wc = ((counts.max(axis=0) + 127) // 128).astype(np.int64)

    ntiles = int(tiles_wc.sum())
    slots = ntiles * 128

    nbatch = (NWIN + WB - 1) // WB
    calls = []
    for b in range(nbatch):
        wlo, whi = b * WB, min((b + 1) * WB, NWIN)
        for ch in range(NCHUNK):
            calls.append((b, ch, int(tiles_wc[wlo:whi, ch].sum())))
    max_call_tiles = max(cl[2] for cl in calls)

    wc_start = np.zeros((NWIN, NCHUNK), np.int64)
    acc = 0
    for wdx in range(NWIN):
        for ch in range(NCHUNK):
            wc_start[wdx, ch] = acc
            acc += int(tiles_wc[wdx, ch])

    per_core = []
    for c in range(NCORES):
        d = cores[c]
        meta = np.zeros((slots, 4), np.float32)   # dst_slot, w, t, pad
        gidx = np.zeros(slots, np.int32)
        pos = 0
        key = d["win"] * NCHUNK + d["chunk"]
        bounds = np.searchsorted(key, np.arange(NWIN * NCHUNK + 1))
        for wdx in range(NWIN):
            for ch in range(NCHUNK):
                k = wdx * NCHUNK + ch
                s, e = bounds[k], bounds[k + 1]
                n = e - s
                T = int(tiles_wc[wdx, ch])
                assert n <= T * 128
                meta[pos:pos + n, 0] = d["slot"][s:e]
                meta[pos:pos + n, 1] = d["w"][s:e]
                meta[pos:pos + n, 2] = d["t"][s:e]
                gidx[pos:pos + n] = d["src"][s:e]
                gidx[pos + n:pos + T * 128] = ch * CHUNK_ROWS  # valid pad row
                pos += T * 128
        assert pos == slots

        meta_blocks = np.zeros((len(calls), 128, max_call_tiles * 4),
                               np.float32)
        idx_blocks = np.full((len(calls), 128, max_call_tiles * 8),
                             -1, np.int16)
        for ci, (b, ch, tc) in enumerate(calls):
            if tc == 0:
                continue
            wlo, whi = b * WB, min((b + 1) * WB, NWIN)
            blk = np.concatenate(
                [meta[wc_start[wdx, ch] * 128:
                      (wc_start[wdx, ch] + int(tiles_wc[wdx, ch])) * 128]
                 for wdx in range(wlo, whi)], axis=0)
            gi = np.concatenate(
                [gidx[wc_start[wdx, ch] * 128:
                      (wc_start[wdx, ch] + int(tiles_wc[wdx, ch])) * 128]
                 for wdx in range(wlo, whi)], axis=0) - ch * CHUNK_ROWS
            assert blk.shape[0] == tc * 128
            m = blk.reshape(tc, 128, 4).transpose(1, 0, 2).reshape(128, tc * 4)
            meta_blocks[ci, :, :tc * 4] = m
            assert gi.min() >= 0 and gi.max() < CHUNK_ROWS
            # dma_gather idx layout: idx j at [partition j%16, column j//16],
            # replicated across the 8 Q7 core groups
            cols = tc * 128 // 16
            lay = gi.astype(np.int16).reshape(cols, 16).T
            idx_blocks[ci, :, :cols] = np.tile(lay, (8, 1))

        xsh = np.zeros((SHARD_ROWS, cfg["IN_CH"]), np.float32)
        own_lo = np.arange(c * PCH, (c + 1) * PCH)
        xsh[:PCH] = x[own_lo]
        xsh[cfg["WPH"] * 128:cfg["WPH"] * 128 + PCH] = x[own_lo + HALF]
        xT = np.ascontiguousarray(xsh.T)

        per_core.append(dict(meta=meta_blocks, idx=idx_blocks, xT=xT))

    structure = dict(tiles_wc=tiles_wc, calls=calls, ntiles=ntiles,
                     max_call_tiles=max_call_tiles, nbatch=nbatch)
    return structure, per_core


def _tile_iter(cfg, structure):
    """Yields (call_index, tile_within_call) in (window, chunk, tile) order."""
    tiles_wc = structure["tiles_wc"]
    calls = structure["calls"]
    call_idx = {(b, ch): i for i, (b, ch, _) in enumerate(calls)}
    cursor = [0] * len(calls)
    for wdx in range(cfg["NWIN"]):
        b = wdx // cfg["WB"]
        for ch in range(cfg["NCHUNK"]):
            ci = call_idx[(b, ch)]
            for _ in range(int(structure["tiles_wc"][wdx, ch])):
                yield ci, cursor[ci]
                cursor[ci] += 1


# ---------------------------------------------------------------------------
# Numpy emulation of the exact device algorithm (debug/validation)
# ---------------------------------------------------------------------------
def emulate(cfg, structure, per_core, W1, b1, a1, W2, b2, a2):
    NWIN, NCHUNK, NCORES = cfg["NWIN"], cfg["NCHUNK"], cfg["NCORES"]
    WPH, PCH, C1, C2 = cfg["WPH"], cfg["PCH"], cfg["C1"], cfg["C2"]
    tiles_wc = structure["tiles_wc"]
    calls = structure["calls"]
    iota = np.arange(128, dtype=np.float32)

    yw1_shards, dinv_all = [], []
    for c in range(NCORES):
        meta = per_core[c]["meta"]
        dinv = np.zeros((NWIN, 128, 2), np.float32)
        ti = _tile_iter(cfg, structure)
        for wdx in range(NWIN):
            deg = np.zeros((128, 2), np.float32)
            for ch in range(NCHUNK):
                for _ in range(int(tiles_wc[wdx, ch])):
                    ci, tloc = next(ti)
                    m = meta[ci][:, tloc * 4:tloc * 4 + 4]
                    onehot = (iota[None, :] == m[:, 0:1])
                    deg += onehot.T.astype(np.float32) @ m[:, 1:3]
            dinv[wdx] = 1.0 / np.sqrt(np.maximum(deg, 1e-12))
        dinv_all.append(dinv)
        xT = per_core[c]["xT"]
        yw1_shards.append((xT.T @ W1) * dinv[:, :, 0].reshape(-1, 1))
    yw1_full = np.concatenate(yw1_shards, 0)

    yw2_shards = []
    for c in range(NCORES):
        meta, idxb = per_core[c]["meta"], per_core[c]["idx"]
        dinv = dinv_all[c]
        yw2 = np.zeros((cfg["SHARD_ROWS"], C2), np.float32)
        ti = _tile_iter(cfg, structure)
        gathered = _emu_gather(cfg, idxb, calls, yw1_full, C1)
        for wdx in range(NWIN):
            acc = np.zeros((128, C1), np.float32)
            for ch in range(NCHUNK):
                for _ in range(int(tiles_wc[wdx, ch])):
                    ci, tloc = next(ti)
                    m = meta[ci][:, tloc * 4:tloc * 4 + 4]
                    S = (iota[None, :] == m[:, 0:1]) * m[:, 1:2]
                    G = gathered[ci][:, tloc * C1:(tloc + 1) * C1]
                    acc += S.T @ G
            z = acc * dinv[wdx, :, 0:1] + b1[None, :]
            h1 = np.maximum(z, 0) + a1[None, :] * np.minimum(z, 0)
            yw2[wdx * 128:(wdx + 1) * 128] = (h1 @ W2) * dinv[wdx, :, 1:2]
        yw2_shards.append(yw2)
    yw2_full = np.concatenate(yw2_shards, 0)

    outs = []
    for c in range(NCORES):
        meta, idxb = per_core[c]["meta"], per_core[c]["idx"]
        dinv = dinv_all[c]
        ti = _tile_iter(cfg, structure)
        gathered = _emu_gather(cfg, idxb, calls, yw2_full, C2)
        h2 = np.zeros((NWIN, 128, C2), np.float32)
        for wdx in range(NWIN):
            acc = np.zeros((128, C2), np.float32)
            for ch in range(NCHUNK):
                for _ in range(int(tiles_wc[wdx, ch])):
                    ci, tloc = next(ti)
                    m = meta[ci][:, tloc * 4:tloc * 4 + 4]
                    S = (iota[None, :] == m[:, 0:1]) * m[:, 2:3]
                    G = gathered[ci][:, tloc * C2:(tloc + 1) * C2]
                    acc += S.T @ G
            z = acc * dinv[wdx, :, 1:2] + b2[None, :]
            h2[wdx] = np.maximum(z, 0) + a2[None, :] * np.minimum(z, 0)
        lo = h2[:WPH].reshape(-1, C2)[:PCH]
        hi = h2[WPH:].reshape(-1, C2)[:PCH]
        outs.append((lo + hi) * 0.5)
    return np.concatenate(outs, 0)


def _emu_gather(cfg, idx_blocks, calls, table, width):
    out = []
    for ci, (b, ch, tc) in enumerate(calls):
        g = np.zeros((128, max(tc, 1) * width), np.float32)
        if tc:
            cols = tc * 128 // 16
            lay = idx_blocks[ci][:16, :cols]
            idx = lay.T.reshape(-1).astype(np.int64) + ch * cfg["CHUNK_ROWS"]
            rows = table[idx]
            g = rows.reshape(tc, 128, width).transpose(1, 2, 0).transpose(
                0, 2, 1).reshape(128, tc * width)
        out.append(g)
    return out


# ---------------------------------------------------------------------------
# Bass kernel builder
# ---------------------------------------------------------------------------
def build_bass(cfg, structure):
    import os

    import concourse.bass as bass
    import concourse.tile as tile
    from concourse import bacc as bacc_mod
    from concourse import mybir

    stop = os.environ.get("GCN_STOP", "full")  # A | B | C | full

    NWIN, NCHUNK, WB, WPH = cfg["NWIN"], cfg["NCHUNK"], cfg["WB"], cfg["WPH"]
    C1, C2 = cfg["C1"], cfg["C2"]
    SHARD_ROWS, TABLE_ROWS, CHUNK_ROWS = (cfg["SHARD_ROWS"],
                                          cfg["TABLE_ROWS"],
                                          cfg["CHUNK_ROWS"])
    tiles_wc = structure["tiles_wc"]
    calls = structure["calls"]
    mct = structure["max_call_tiles"]
    ncalls = len(calls)
    f32 = mybir.dt.float32
    AF = mybir.ActivationFunctionType
    OP = mybir.AluOpType

    # Bacc (not plain Bass): finalize() runs the TRN2 legalization passes
    # (sync-wait splitting, custom-ISA codegen, library load insertion).
    nc = bacc_mod.Bacc(num_devices=cfg["NCORES"])

    # I/O
    meta_d = nc.declare_dram_parameter("meta", [ncalls * 128, mct * 4], f32,
                                       isOutput=False)
    idx_d = nc.declare_dram_parameter("idx", [ncalls * 128, mct * 8],
                                      mybir.dt.int16, isOutput=False)
    xT_d = nc.declare_dram_parameter("xT", [128, SHARD_ROWS], f32,
                                     isOutput=False)
    W1_d = nc.declare_dram_parameter("W1", [128, C1], f32, isOutput=False)
    W2_d = nc.declare_dram_parameter("W2", [C1, C2], f32, isOutput=False)
    b1_d = nc.declare_dram_parameter("b1r", [128, C1], f32, isOutput=False)
    a1_d = nc.declare_dram_parameter("a1r", [128, C1], f32, isOutput=False)
    b2_d = nc.declare_dram_parameter("b2r", [128, C2], f32, isOutput=False)
    a2_d = nc.declare_dram_parameter("a2r", [128, C2], f32, isOutput=False)
    iota_d = nc.declare_dram_parameter("iota", [128, 128], f32, isOutput=False)
    ident_d = nc.declare_dram_parameter("ident", [128, 128], f32,
                                        isOutput=False)
    out_d = nc.declare_dram_parameter("out", [WPH * 128, C2], f32,
                                      isOutput=True)

    rg = [list(range(cfg["NCORES"]))]

    with tile.TileContext(nc, num_cores=cfg["NCORES"]) as tc_:
        with (
            tc_.tile_pool(name="const", bufs=1) as constp,
            tc_.tile_pool(name="dinv", bufs=1) as dinvp,
            tc_.tile_pool(name="meta", bufs=6) as metap,
            tc_.tile_pool(name="idx", bufs=6) as idxp,
            tc_.tile_pool(name="g", bufs=6) as gp,
            tc_.tile_pool(name="s", bufs=4) as sp,
            tc_.tile_pool(name="ev", bufs=3) as evp,
            tc_.tile_pool(name="stash", bufs=1) as stashp,
            tc_.tile_pool(name="xtp", bufs=3) as xtp,
            tc_.tile_pool(name="degps", bufs=2, space="PSUM") as degps,
            tc_.tile_pool(name="winps", bufs=2, space="PSUM") as winps,
            tc_.tile_pool(name="tps", bufs=2, space="PSUM") as tps,
            tc_.tile_pool(name="y2ps", bufs=2, space="PSUM") as y2ps,
            tc_.tile_pool(name="dram", bufs=1, space="DRAM") as dramp,
        ):
            # ---- constants into SBUF
            iota_sb = constp.tile([128, 128], f32, name="iota_sb")
            ident_sb = constp.tile([128, 128], f32, name="ident_sb")
            W1_sb = constp.tile([128, C1], f32, name="W1_sb")
            W2_sb = constp.tile([C1, C2], f32, name="W2_sb")
            b1_sb = constp.tile([128, C1], f32, name="b1_sb")
            a1_sb = constp.tile([128, C1], f32, name="a1_sb")
            b2_sb = constp.tile([128, C2], f32, name="b2_sb")
            a2_sb = constp.tile([128, C2], f32, name="a2_sb")
            for sb, dr in ((iota_sb, iota_d), (ident_sb, ident_d),
                           (W1_sb, W1_d), (W2_sb, W2_d), (b1_sb, b1_d),
                           (a1_sb, a1_d), (b2_sb, b2_d), (a2_sb, a2_d)):
                nc.sync.dma_start(out=sb, in_=dr[:, :])

            dinv_sb = dinvp.tile([128, NWIN * 2], f32, name="dinv_sb")

            # DRAM scratch
            yw1_shard = dramp.tile([SHARD_ROWS, C1], f32, name="yw1_shard")
            yw1_full = dramp.tile([TABLE_ROWS, C1], f32, name="yw1_full",
                                  addr_space="Shared")
            yw2_shard = dramp.tile([SHARD_ROWS, C2], f32, name="yw2_shard")
            yw2_full = dramp.tile([TABLE_ROWS, C2], f32, name="yw2_full",
                                  addr_space="Shared")

            call_of = {(b, ch): i for i, (b, ch, _) in enumerate(calls)}

            # one Pool register per distinct num_idxs value (to_reg allocates
            # a fresh register per call and the register file is small)
            _nreg_cache = {}

            def nreg(v):
                if v not in _nreg_cache:
                    _nreg_cache[v] = nc.gpsimd.to_reg(v)
                return _nreg_cache[v]

            # ================= pass A: degrees -> dinv =================
            ab = os.environ.get("GCN_AB", "")
            if ab == "nodve":
                s_const = constp.tile([128, 128], f32, name="s_const")
                nc.vector.tensor_copy(out=s_const, in_=iota_sb)
            if ab in ("nomm", "noevict"):
                nc.vector.memset(dinv_sb[:, :], 1.0)
            meta_tiles = {}

            def load_meta(ci):
                t = metap.tile([128, mct * 4], f32, tag="meta")
                nc.sync.dma_start(out=t,
                                  in_=meta_d[ci * 128:(ci + 1) * 128, :])
                return t

            cursor = [0] * ncalls
            for b in range(structure["nbatch"]):
                for ch in range(NCHUNK):
                    ci = call_of[(b, ch)]
                    if calls[ci][2]:
                        meta_tiles[ci] = load_meta(ci)
                wlo = b * WB
                whi = min(wlo + WB, NWIN)
                for wdx in range(wlo, whi):
                    ntile_w = int(tiles_wc[wdx].sum())
                    if ab == "wide":
                        deg_ps = winps.tile([128, 128], f32, tag="win")
                    else:
                        deg_ps = degps.tile([128, 2], f32, tag="deg")
                    k = 0
                    for ch in range(NCHUNK):
                        ci = call_of[(b, ch)]
                        for _ in range(int(tiles_wc[wdx, ch])):
                            tloc = cursor[ci]
                            cursor[ci] += 1
                            m = meta_tiles[ci]
                            if ab == "nodve":
                                s_t = s_const
                            else:
                                s_t = sp.tile([128, 128], f32, tag="s")
                                nc.vector.tensor_scalar(
                                    out=s_t, in0=iota_sb,
                                    scalar1=(5.0 if ab == "imm" else
                                             m[:, 4 * tloc:4 * tloc + 1]),
                                    scalar2=None, op0=OP.is_equal)
                            if ab == "nomm":
                                pass
                            elif ab == "wide":
                                nc.tensor.matmul(
                                    out=deg_ps, lhsT=s_t, rhs=iota_sb,
                                    start=(k == 0), stop=(k == ntile_w - 1))
                            else:
                                nc.tensor.matmul(
                                    out=deg_ps, lhsT=s_t,
                                    rhs=m[:, 4 * tloc + 1:4 * tloc + 3],
                                    start=(k == 0), stop=(k == ntile_w - 1))
                            k += 1
                    # dinv = 1/sqrt(deg); deg >= 1 (self loop).  Rsqrt is
                    # banned in bass (accuracy) -> reciprocal then sqrt.
                    if ab in ("nomm", "noevict"):
                        continue
                    rec_t = evp.tile([128, 2], f32, tag="rec")
                    nc.vector.reciprocal(out=rec_t, in_=deg_ps[:, 0:2])
                    nc.scalar.activation(
                        out=dinv_sb[:, 2 * wdx:2 * wdx + 2], in_=rec_t,
                        func=AF.Sqrt)

            # ================= pass B: yw1 shard + AllGather ============
            if stop == "A":
                nc.sync.dma_start(out=out_d[0:128, :],
                                  in_=dinv_sb[:, 0:C2])
            if stop in ("B", "C0", "C1", "C", "full"):
                for wdx in range(NWIN):
                    xt_t = xtp.tile([128, 128], f32, tag="xt")
                    nc.sync.dma_start(out=xt_t,
                                      in_=xT_d[:, wdx * 128:(wdx + 1) * 128])
                    xw_ps = y2ps.tile([128, C1], f32, tag="y2")
                    nc.tensor.matmul(out=xw_ps, lhsT=xt_t, rhs=W1_sb,
                                     start=True, stop=True)
                    yw_t = evp.tile([128, C1], f32, tag="yw")
                    nc.vector.tensor_scalar(
                        out=yw_t, in0=xw_ps,
                        scalar1=dinv_sb[:, 2 * wdx:2 * wdx + 1],
                        scalar2=None, op0=OP.mult)
                    nc.sync.dma_start(
                        out=yw1_shard[wdx * 128:(wdx + 1) * 128, :], in_=yw_t)

                nc.gpsimd.collective_compute(
                    "AllGather", OP.bypass, replica_groups=rg,
                    ins=[yw1_shard[:, :]], outs=[yw1_full[:, :]])
            if stop == "B":
                t_dbg = evp.tile([128, C2], f32, tag="dbg")
                nc.sync.dma_start(out=t_dbg, in_=yw1_full[0:128, 0:C2])
                nc.sync.dma_start(out=out_d[0:128, :], in_=t_dbg)

            # ============ pass C: layer-1 messages, fused yw2 ===========
            def msg_pass(table, width, wcol, dcol, b_sb, a_sb, out_cb):
                """wcol: meta column for edge weight (1=w, 2=type);
                dcol: dinv column (0 or 1) used at eviction;
                out_cb(wdx, h_tile): consume the [128, width] result."""
                cursor = [0] * ncalls
                for b in range(structure["nbatch"]):
                    g_tiles = {}
                    for ch in range(NCHUNK):
                        ci = call_of[(b, ch)]
                        tcn = calls[ci][2]
                        if not tcn:
                            continue
                        meta_tiles[ci] = load_meta(ci)
                        it = idxp.tile([128, mct * 8], mybir.dt.int16,
                                       tag="idx")
                        nc.sync.dma_start(
                            out=it, in_=idx_d[ci * 128:(ci + 1) * 128, :])
                        g_t = gp.tile([128, mct * C1], f32, tag="g")
                        if not os.environ.get("GCN_NOGATHER"):
                            nc.gpsimd.dma_gather(
                                out_ap=g_t[:, :tcn * width].rearrange(
                                    "p (t e) -> p t e", e=width),
                                in_ap=table[ch * CHUNK_ROWS:
                                            (ch + 1) * CHUNK_ROWS, :],
                                idxs_ap=it[:, :tcn * 8],
                                num_idxs=tcn * 128,
                                num_idxs_reg=nreg(tcn * 128),
                                elem_size=width,
                                # single_packet=True breaks for calls over
                                # ~384 indices (HW-bisected)
                                single_packet=False)
                        else:
                            nc.vector.tensor_copy(out=g_t[:, 0:128],
                                                  in_=iota_sb)
                        g_tiles[ch] = g_t
                    wlo = b * WB
                    whi = min(wlo + WB, NWIN)
                    for wdx in range(wlo, whi):
                        ntile_w = int(tiles_wc[wdx].sum())
                        h_ps = winps.tile([128, width], f32, tag="win")
                        k = 0
                        for ch in range(NCHUNK):
                            ci = call_of[(b, ch)]
                            for _ in range(int(tiles_wc[wdx, ch])):
                                tloc = cursor[ci]
                                cursor[ci] += 1
                                m = meta_tiles[ci]
                                s_t = sp.tile([128, 128], f32, tag="s")
                                nc.vector.tensor_scalar(
                                    out=s_t, in0=iota_sb,
                                    scalar1=m[:, 4 * tloc:4 * tloc + 1],
                                    scalar2=m[:, 4 * tloc + wcol:
                                              4 * tloc + wcol + 1],
                                    op0=OP.is_equal, op1=OP.mult)
                                nc.tensor.matmul(
                                    out=h_ps, lhsT=s_t,
                                    rhs=g_tiles[ch][:, tloc * width:
                                                    (tloc + 1) * width],
                                    start=(k == 0), stop=(k == ntile_w - 1))
                                k += 1
                        # evict: z = psum * dinv + b ; h = prelu(z, a)
                        if os.environ.get("GCN_NOEVICT"):
                            h_t = evp.tile([128, width], f32, tag="h")
                            nc.vector.tensor_copy(out=h_t, in_=h_ps)
                            out_cb(wdx, h_t)
                            continue
                        dv = dinv_sb[:, 2 * wdx + dcol:2 * wdx + dcol + 1]
                        z_t = evp.tile([128, width], f32, tag="z")
                        nc.vector.scalar_tensor_tensor(
                            out=z_t, in0=h_ps, scalar=dv, in1=b_sb,
                            op0=OP.mult, op1=OP.add)
                        mn_t = evp.tile([128, width], f32, tag="mn")
                        nc.vector.tensor_scalar(
                            out=mn_t, in0=z_t, scalar1=0.0, scalar2=None,
                            op0=OP.min)
                        am_t = evp.tile([128, width], f32, tag="am")
                        nc.vector.tensor_tensor(out=am_t, in0=mn_t, in1=a_sb,
                                                op=OP.mult)
                        h_t = evp.tile([128, width], f32, tag="h")
                        nc.vector.scalar_tensor_tensor(
                            out=h_t, in0=z_t, scalar=0.0, in1=am_t,
                            op0=OP.max, op1=OP.add)
                        out_cb(wdx, h_t)

            def l1_out(wdx, h_t):
                # fused layer-2 dense: yw2 = (h1 @ W2) * dinv2
                t_ps = tps.tile([128, 128], f32, tag="tp")
                nc.tensor.transpose(out=t_ps, in_=h_t, identity=ident_sb)
                h1T = evp.tile([128, 128], f32, tag="h1T")
                nc.vector.tensor_copy(out=h1T, in_=t_ps)
                y2_ps = y2ps.tile([128, C2], f32, tag="y2")
                nc.tensor.matmul(out=y2_ps, lhsT=h1T, rhs=W2_sb,
                                 start=True, stop=True)
                yw2_t = evp.tile([128, C2], f32, tag="yw2")
                nc.vector.tensor_scalar(
                    out=yw2_t, in0=y2_ps,
                    scalar1=dinv_sb[:, 2 * wdx + 1:2 * wdx + 2],
                    scalar2=None, op0=OP.mult)
                nc.sync.dma_start(
                    out=yw2_shard[wdx * 128:(wdx + 1) * 128, :], in_=yw2_t)

            def l1_out_nofuse(wdx, h_t):
                nc.sync.dma_start(
                    out=yw2_shard[wdx * 128:(wdx + 1) * 128, :],
                    in_=h_t[:, :C2])

            if stop in ("C0",):
                msg_pass(yw1_full, C1, 1, 0, b1_sb, a1_sb, l1_out_nofuse)
            if stop in ("C1",):
                msg_pass(yw1_full, C1, 1, 0, b1_sb, a1_sb, l1_out)
            if stop in ("C", "full"):
                msg_pass(yw1_full, C1, 1, 0, b1_sb, a1_sb, l1_out)

                nc.gpsimd.collective_compute(
                    "AllGather", OP.bypass, replica_groups=rg,
                    ins=[yw2_shard[:, :]], outs=[yw2_full[:, :]])
            if stop == "C":
                t_dbg = evp.tile([128, C2], f32, tag="dbg")
                nc.sync.dma_start(out=t_dbg, in_=yw2_full[0:128, :])
                nc.sync.dma_start(out=out_d[0:128, :], in_=t_dbg)
            if stop in ("C0", "C1"):
                t_dbg = evp.tile([128, C2], f32, tag="dbg")
                nc.sync.dma_start(out=t_dbg, in_=yw2_shard[0:128, :])
                nc.sync.dma_start(out=out_d[0:128, :], in_=t_dbg)

            # ============ pass E: layer-2 messages + combine ============
            stash = stashp.tile([128, WPH * C2], f32, name="h2lo")

            def l2_out(wdx, h_t):
                if wdx < WPH:
                    nc.vector.tensor_copy(
                        out=stash[:, wdx * C2:(wdx + 1) * C2], in_=h_t)
                else:
                    w2 = wdx - WPH
                    cmb = evp.tile([128, C2], f32, tag="cmb")
                    nc.vector.tensor_tensor(
                        out=cmb, in0=h_t,
                        in1=stash[:, w2 * C2:(w2 + 1) * C2], op=OP.add)
                    o_t = evp.tile([128, C2], f32, tag="o")
                    nc.vector.tensor_scalar(
                        out=o_t, in0=cmb, scalar1=0.5, scalar2=None,
                        op0=OP.mult)
                    nc.sync.dma_start(
                        out=out_d[w2 * 128:(w2 + 1) * 128, :], in_=o_t)

            if stop == "full":
                msg_pass(yw2_full, C2, 2, 1, b2_sb, a2_sb, l2_out)

    nc.finalize()
    return nc


# ---------------------------------------------------------------------------
# Host driver
# ---------------------------------------------------------------------------
def make_in_maps(cfg, structure, per_core, W1, b1, a1, W2, b2, a2):
    mct = structure["max_call_tiles"]
    ncalls = len(structure["calls"])
    iota = np.tile(np.arange(128, dtype=np.float32), (128, 1))
    ident = np.eye(128, dtype=np.float32)
    consts = dict(
        W1=np.ascontiguousarray(W1, np.float32),
        W2=np.ascontiguousarray(W2, np.float32),
        b1r=np.tile(b1.astype(np.float32), (128, 1)),
        a1r=np.tile(a1.astype(np.float32), (128, 1)),
        b2r=np.tile(b2.astype(np.float32), (128, 1)),
        a2r=np.tile(a2.astype(np.float32), (128, 1)),
        iota=np.ascontiguousarray(iota),
        ident=ident,
    )
    in_maps = []
    for c in range(cfg["NCORES"]):
        pc = per_core[c]
        in_maps.append(dict(
            meta=pc["meta"].reshape(ncalls * 128, mct * 4),
            idx=pc["idx"].reshape(ncalls * 128, mct * 8),
            xT=pc["xT"],
            **consts,
        ))
    return in_maps


def assemble_out(cfg, outs):
    """outs: list per core of the 'out' array [WPH*128, C2]."""
    parts = [o[:cfg["PCH"]] for o in outs]
    return np.ascontiguousarray(np.concatenate(parts, 0), dtype=np.float32)


LAST_EXEC_NS = None


def _trivial_nc(ncores):
    """A minimal bass kernel for dispatch-overhead calibration."""
    from concourse import bacc as bacc_mod
    from concourse import mybir
    import concourse.tile as tile

    f32 = mybir.dt.float32
    nc = bacc_mod.Bacc(num_devices=ncores)
    a = nc.declare_dram_parameter("a", [128, 128], f32, isOutput=False)
    o = nc.declare_dram_parameter("o", [128, 128], f32, isOutput=True)
    with tile.TileContext(nc, num_cores=ncores) as tc:
        with tc.tile_pool(name="p", bufs=2) as p:
            t = p.tile([128, 128], f32)
            nc.sync.dma_start(out=t, in_=a[:, :])
            nc.sync.dma_start(out=o[:, :], in_=t)
    nc.finalize()
    return nc


def _time_kernel(nc, in_maps, n_cores, iters=5, reps=10, n_iters=None):
    """Best-of-N wall time of one dispatched execution (device put inputs,
    block_until_ready).  Subtract a trivial-kernel baseline for HW time.
    If n_iters is given, run the kernel n_iters times inside one dispatch
    (plus a 1-iter program) and return (T(n) - T(1)) / (n - 1), which
    cancels dispatch overhead far more precisely."""
    import time

    import jax
    import numpy as np
    from jax.experimental.shard_map import shard_map
    from jax.sharding import Mesh, PartitionSpec

    from concourse import bass2jax, mybir

    bass2jax.install_neuronx_cc_hook()
    partition_name = (nc.partition_id_tensor.name
                      if nc.partition_id_tensor else None)
    in_names, out_names, out_avals, zero_outs = [], [], [], []
    for alloc in nc.m.functions[0].allocations:
        if not isinstance(alloc, mybir.MemoryLocationSet):
            continue
        name = alloc.memorylocations[0].name
        if alloc.kind == "ExternalInput":
            if name != partition_name:
                in_names.append(name)
        elif alloc.kind == "ExternalOutput":
            out_names.append(name)
            shape = tuple(alloc.tensor_shape)
            dtype = mybir.dt.np(alloc.dtype)
            out_avals.append(jax.core.ShapedArray(shape, dtype))
            zero_outs.append(np.zeros(shape, dtype))
    n_params = len(in_names)
    all_in_names = list(in_names) + list(out_names)
    if partition_name is not None:
        all_in_names.append(partition_name)

    n_outs_ = len(out_names)

    def make_body(n):
        # each iteration gets its own zero-buffer parameters: operands stay
        # top-level parameters (hook requirement) and differ across
        # iterations (no CSE); the effectful primitive keeps them ordered.
        def _body(*args):
            ins = list(args[:n_params])
            outs = None
            for i in range(n):
                zs = list(args[n_params + i * n_outs_:
                               n_params + (i + 1) * n_outs_])
                operands = ins + zs
                if partition_name is not None:
                    operands.append(bass2jax.partition_id_tensor())
                outs = bass2jax._bass_exec_p.bind(
                    *operands,
                    out_avals=tuple(out_avals),
                    in_names=tuple(all_in_names),
                    out_names=tuple(out_names),
                    lowering_input_output_aliases=(),
                    sim_require_finite=False,
                    sim_require_nnan=False,
                    nc=nc,
                )
            return tuple(outs)
        return _body

    devices = jax.devices()[:n_cores]
    mesh = Mesh(np.asarray(devices), ("core",))

    per_core = [[np.asarray(m[name]) for name in in_names] for m in in_maps]
    concat_in = [np.concatenate([per_core[c][i] for c in range(n_cores)], 0)
                 for i in range(n_params)]
    concat_zeros = [np.zeros((n_cores * z.shape[0], *z.shape[1:]), z.dtype)
                    for z in zero_outs]
    dev_in = [jax.device_put(a) for a in concat_in]
    dev_zero = [jax.device_put(a) for a in concat_zeros]

    n = 1
    in_specs = (PartitionSpec("core"),) * (n_params + n * n_outs_)
    out_specs = (PartitionSpec("core"),) * n_outs_
    fn = jax.jit(shard_map(make_body(n), mesh=mesh, in_specs=in_specs,
                           out_specs=out_specs, check_rep=False),
                 keep_unused=True)
    args = dev_in + dev_zero * n
    out = fn(*args)  # compile + warm
    jax.block_until_ready(out)

    def measure(k):
        # k dispatches issued back-to-back, one final block: host RTT
        # overlaps device execution, so slope over k isolates device time.
        times = []
        for _ in range(reps):
            t0 = time.perf_counter()
            out = None
            for _ in range(k):
                out = fn(*args)
            jax.block_until_ready(out)
            times.append(time.perf_counter() - t0)
        times.sort()
        print(f"  timing(k={k}): wall times ms = "
              f"{[f'{t*1e3:.2f}' for t in times[:8]]}")
        return times[0] * 1e9

    if n_iters is not None and n_iters > 1:
        t_hi = measure(n_iters)
        t_lo = measure(1)
        per = (t_hi - t_lo) / (n_iters - 1)
        print(f"  timing: T({n_iters})={t_hi/1e6:.3f} ms, T(1)={t_lo/1e6:.3f}"
              f" ms -> per-dispatch {per/1e6:.3f} ms")
        return per
    return measure(1)


def kernel(**inputs):
    global LAST_EXEC_NS
    import os

    cfg = CFG
    x = np.asarray(inputs["x"], np.float32)
    W1 = np.asarray(inputs["W1"], np.float32)
    b1 = np.asarray(inputs["b1"], np.float32)
    a1 = np.asarray(inputs["a1"], np.float32)
    W2 = np.asarray(inputs["W2"], np.float32)
    b2 = np.asarray(inputs["b2"], np.float32)
    a2 = np.asarray(inputs["a2"], np.float32)

    structure, per_core = prep(cfg, x, inputs["edge_index"],
                               inputs["edge_weight"], inputs["edge_type"])

    if os.environ.get("GCN_EMULATE"):
        return emulate(cfg, structure, per_core, W1, b1, a1, W2, b2, a2)

    from concourse.bass_utils import run_bass_kernel_spmd

    nc = build_bass(cfg, structure)
    in_maps = make_in_maps(cfg, structure, per_core, W1, b1, a1, W2, b2, a2)
    res = run_bass_kernel_spmd(
        nc, in_maps, core_ids=list(range(cfg["NCORES"])))
    LAST_EXEC_NS = res.exec_time_ns
    if os.environ.get("GCN_TIME"):
        n_it = int(os.environ.get("GCN_ITERS", "9"))
        main_ns = _time_kernel(nc, in_maps, cfg["NCORES"], n_iters=n_it)
        LAST_EXEC_NS = main_ns
    return assemble_out(cfg, [res.results[c]["out"]
                              for c in range(cfg["NCORES"])])



# revision 11
# speedup vs baseline: 1.4050x; 1.2429x over previous
"""Trainium2 Bass kernel for nn_DoubleLayeredEncoder (2-layer GCN, N=100k, E=1.6M).

Strategy (8 NeuronCores, SPMD, one NEFF):
  - Each core owns 6250 "lo" nodes [6250c, 6250(c+1)) and the paired 6250 "hi"
    nodes [50000+6250c, ...), so the final (n1+n2)/2 is core-local.
  - Edges are assigned to the core owning dst, sorted into 98 windows of 128
    dst slots, and within each window grouped by src chunk (4 chunks of the
    gather table, since dma_gather indices are int16).
  - Per 128-edge tile: one DVE tensor_scalar builds the one-hot selection
    matrix S[e,d] = (iota[d] == dst_slot[e]) * w[e]; the tensor engine
    accumulates psum[d,f] += S.T @ G where G = gathered source rows.
  - Source rows come from yw = dinv * (x @ W) tables: each core computes its
    shard (deg -> rsqrt -> scale), then an AllGather makes the full table
    available for dma_gather.  dinv[dst] is applied at window eviction.
  - Layer-2 dense matmul (h1 @ W2) is fused into layer-1 window eviction via
    a PE transpose.
"""

import math

import numpy as np


# ---------------------------------------------------------------------------
# Config
# ---------------------------------------------------------------------------
def make_cfg(n=100000, ncores=8, nchunk=4, wb=4):
    c = {}
    c["N"] = n
    c["IN_CH"] = 128
    c["C1"] = 128
    c["C2"] = 64
    c["NCORES"] = ncores
    c["HALF"] = n // 2
    c["PCH"] = c["HALF"] // ncores            # nodes per core per half
    c["OWN"] = 2 * c["PCH"]
    c["WPH"] = (c["PCH"] + 127) // 128        # windows per half
    c["NWIN"] = 2 * c["WPH"]
    c["SHARD_ROWS"] = c["NWIN"] * 128
    c["TABLE_ROWS"] = ncores * c["SHARD_ROWS"]
    c["NCHUNK"] = nchunk
    assert c["TABLE_ROWS"] % nchunk == 0
    c["CHUNK_ROWS"] = c["TABLE_ROWS"] // nchunk
    assert c["CHUNK_ROWS"] <= 32768, "dma_gather idx is int16"
    c["WB"] = wb
    return c


CFG = make_cfg()


def _row_of_node(c, j):
    """Row of node j in the allgathered (rank-block-concatenated) tables."""
    j = np.asarray(j)
    lo = j < c["HALF"]
    core = np.where(lo, j // c["PCH"], (j - c["HALF"]) // c["PCH"])
    pos = np.where(lo, j - core * c["PCH"], j - c["HALF"] - core * c["PCH"])
    return core * c["SHARD_ROWS"] + np.where(lo, pos, c["WPH"] * 128 + pos)


# ---------------------------------------------------------------------------
# Host-side prep: per-core edge tiles, metadata, gather indices
# ---------------------------------------------------------------------------
def prep(cfg, x, edge_index, edge_weight, edge_type):
    NCORES, NWIN, NCHUNK, WB = (cfg["NCORES"], cfg["NWIN"], cfg["NCHUNK"],
                                cfg["WB"])
    CHUNK_ROWS, SHARD_ROWS, PCH, HALF = (cfg["CHUNK_ROWS"], cfg["SHARD_ROWS"],
                                         cfg["PCH"], cfg["HALF"])
    src = np.asarray(edge_index[0], dtype=np.int64)
    dst = np.asarray(edge_index[1], dtype=np.int64)
    w = np.asarray(edge_weight, dtype=np.float32)
    t = np.asarray(edge_type, dtype=np.float32)

    src_row = _row_of_node(cfg, src).astype(np.int32)
    dst_row = _row_of_node(cfg, dst).astype(np.int32)
    core_of_edge = dst_row // SHARD_ROWS

    cores = []
    for c in range(NCORES):
        sel = core_of_edge == c
        e_src = src_row[sel]
        e_dstloc = dst_row[sel] - c * SHARD_ROWS
        e_w = w[sel]
        e_t = t[sel]
        # self loops (weight 1 in both layers) for the real owned nodes
        own_lo = np.arange(c * PCH, (c + 1) * PCH)
        own = np.concatenate([own_lo, own_lo + HALF])
        sl_row = _row_of_node(cfg, own).astype(np.int32)
        e_src = np.concatenate([e_src, sl_row])
        e_dstloc = np.concatenate([e_dstloc, sl_row - c * SHARD_ROWS])
        e_w = np.concatenate([e_w, np.ones(cfg["OWN"], np.float32)])
        e_t = np.concatenate([e_t, np.ones(cfg["OWN"], np.float32)])

        win = e_dstloc >> 7
        slot = e_dstloc & 127
        chunk = e_src // CHUNK_ROWS
        order = np.lexsort((chunk, win))
        cores.append(dict(src=e_src[order], slot=slot[order], w=e_w[order],
                          t=e_t[order], win=win[order], chunk=chunk[order]))

    counts = np.zeros((NCORES, NWIN, NCHUNK), np.int64)
    for c in range(NCORES):
        d = cores[c]
        np.add.at(counts[c], (d["win"], d["chunk"]), 1)
    tiles_wc = ((counts.max(axis=0) + 127) // 128).astype(np.int64)

    ntiles = int(tiles_wc.sum())
    slots = ntiles * 128

    nbatch = (NWIN + WB - 1) // WB
    calls = []
    for b in range(nbatch):
        wlo, whi = b * WB, min((b + 1) * WB, NWIN)
        for ch in range(NCHUNK):
            calls.append((b, ch, int(tiles_wc[wlo:whi, ch].sum())))
    max_call_tiles = max(cl[2] for cl in calls)

    wc_start = np.zeros((NWIN, NCHUNK), np.int64)
    acc = 0
    for wdx in range(NWIN):
        for ch in range(NCHUNK):
            wc_start[wdx, ch] = acc
            acc += int(tiles_wc[wdx, ch])

    per_core = []
    for c in range(NCORES):
        d = cores[c]
        meta = np.zeros((slots, 4), np.float32)   # dst_slot, w, t, pad
        gidx = np.zeros(slots, np.int32)
        pos = 0
        key = d["win"] * NCHUNK + d["chunk"]
        bounds = np.searchsorted(key, np.arange(NWIN * NCHUNK + 1))
        for wdx in range(NWIN):
            for ch in range(NCHUNK):
                k = wdx * NCHUNK + ch
                s, e = bounds[k], bounds[k + 1]
                n = e - s
                T = int(tiles_wc[wdx, ch])
                assert n <= T * 128
                meta[pos:pos + n, 0] = d["slot"][s:e]
                meta[pos:pos + n, 1] = d["w"][s:e]
                meta[pos:pos + n, 2] = d["t"][s:e]
                gidx[pos:pos + n] = d["src"][s:e]
                gidx[pos + n:pos + T * 128] = ch * CHUNK_ROWS  # valid pad row
                pos += T * 128
        assert pos == slots

        meta_blocks = np.zeros((len(calls), 128, max_call_tiles * 4),
                               np.float32)
        idx_blocks = np.full((len(calls), 128, max_call_tiles * 8),
                             -1, np.int16)
        for ci, (b, ch, tc) in enumerate(calls):
            if tc == 0:
                continue
            wlo, whi = b * WB, min((b + 1) * WB, NWIN)
            blk = np.concatenate(
                [meta[wc_start[wdx, ch] * 128:
                      (wc_start[wdx, ch] + int(tiles_wc[wdx, ch])) * 128]
                 for wdx in range(wlo, whi)], axis=0)
            gi = np.concatenate(
                [gidx[wc_start[wdx, ch] * 128:
                      (wc_start[wdx, ch] + int(tiles_wc[wdx, ch])) * 128]
                 for wdx in range(wlo, whi)], axis=0) - ch * CHUNK_ROWS
            assert blk.shape[0] == tc * 128
            m = blk.reshape(tc, 128, 4).transpose(1, 0, 2).reshape(128, tc * 4)
            meta_blocks[ci, :, :tc * 4] = m
            assert gi.min() >= 0 and gi.max() < CHUNK_ROWS
            # dma_gather idx layout: idx j at [partition j%16, column j//16],
            # replicated across the 8 Q7 core groups
            cols = tc * 128 // 16
            lay = gi.astype(np.int16).reshape(cols, 16).T
            idx_blocks[ci, :, :cols] = np.tile(lay, (8, 1))

        xsh = np.zeros((SHARD_ROWS, cfg["IN_CH"]), np.float32)
        own_lo = np.arange(c * PCH, (c + 1) * PCH)
        xsh[:PCH] = x[own_lo]
        xsh[cfg["WPH"] * 128:cfg["WPH"] * 128 + PCH] = x[own_lo + HALF]
        xT = np.ascontiguousarray(xsh.T)

        per_core.append(dict(meta=meta_blocks, idx=idx_blocks, xT=xT))

    structure = dict(tiles_wc=tiles_wc, calls=calls, ntiles=ntiles,
                     max_call_tiles=max_call_tiles, nbatch=nbatch)
    return structure, per_core


def _tile_iter(cfg, structure):
    """Yields (call_index, tile_within_call) in (window, chunk, tile) order."""
    tiles_wc = structure["tiles_wc"]
    calls = structure["calls"]
    call_idx = {(b, ch): i for i, (b, ch, _) in enumerate(calls)}
    cursor = [0] * len(calls)
    for wdx in range(cfg["NWIN"]):
        b = wdx // cfg["WB"]
        for ch in range(cfg["NCHUNK"]):
            ci = call_idx[(b, ch)]
            for _ in range(int(structure["tiles_wc"][wdx, ch])):
                yield ci, cursor[ci]
                cursor[ci] += 1


# ---------------------------------------------------------------------------
# Numpy emulation of the exact device algorithm (debug/validation)
# ---------------------------------------------------------------------------
def emulate(cfg, structure, per_core, W1, b1, a1, W2, b2, a2):
    NWIN, NCHUNK, NCORES = cfg["NWIN"], cfg["NCHUNK"], cfg["NCORES"]
    WPH, PCH, C1, C2 = cfg["WPH"], cfg["PCH"], cfg["C1"], cfg["C2"]
    tiles_wc = structure["tiles_wc"]
    calls = structure["calls"]
    iota = np.arange(128, dtype=np.float32)

    yw1_shards, dinv_all = [], []
    for c in range(NCORES):
        meta = per_core[c]["meta"]
        dinv = np.zeros((NWIN, 128, 2), np.float32)
        ti = _tile_iter(cfg, structure)
        for wdx in range(NWIN):
            deg = np.zeros((128, 2), np.float32)
            for ch in range(NCHUNK):
                for _ in range(int(tiles_wc[wdx, ch])):
                    ci, tloc = next(ti)
                    m = meta[ci][:, tloc * 4:tloc * 4 + 4]
                    onehot = (iota[None, :] == m[:, 0:1])
                    deg += onehot.T.astype(np.float32) @ m[:, 1:3]
            dinv[wdx] = 1.0 / np.sqrt(np.maximum(deg, 1e-12))
        dinv_all.append(dinv)
        xT = per_core[c]["xT"]
        yw1_shards.append((xT.T @ W1) * dinv[:, :, 0].reshape(-1, 1))
    yw1_full = np.concatenate(yw1_shards, 0)

    yw2_shards = []
    for c in range(NCORES):
        meta, idxb = per_core[c]["meta"], per_core[c]["idx"]
        dinv = dinv_all[c]
        yw2 = np.zeros((cfg["SHARD_ROWS"], C2), np.float32)
        ti = _tile_iter(cfg, structure)
        gathered = _emu_gather(cfg, idxb, calls, yw1_full, C1)
        for wdx in range(NWIN):
            acc = np.zeros((128, C1), np.float32)
            for ch in range(NCHUNK):
                for _ in range(int(tiles_wc[wdx, ch])):
                    ci, tloc = next(ti)
                    m = meta[ci][:, tloc * 4:tloc * 4 + 4]
                    S = (iota[None, :] == m[:, 0:1]) * m[:, 1:2]
                    G = gathered[ci][:, tloc * C1:(tloc + 1) * C1]
                    acc += S.T @ G
            z = acc * dinv[wdx, :, 0:1] + b1[None, :]
            h1 = np.maximum(z, 0) + a1[None, :] * np.minimum(z, 0)
            yw2[wdx * 128:(wdx + 1) * 128] = (h1 @ W2) * dinv[wdx, :, 1:2]
        yw2_shards.append(yw2)
    yw2_full = np.concatenate(yw2_shards, 0)

    outs = []
    for c in range(NCORES):
        meta, idxb = per_core[c]["meta"], per_core[c]["idx"]
        dinv = dinv_all[c]
        ti = _tile_iter(cfg, structure)
        gathered = _emu_gather(cfg, idxb, calls, yw2_full, C2)
        h2 = np.zeros((NWIN, 128, C2), np.float32)
        for wdx in range(NWIN):
            acc = np.zeros((128, C2), np.float32)
            for ch in range(NCHUNK):
                for _ in range(int(tiles_wc[wdx, ch])):
                    ci, tloc = next(ti)
                    m = meta[ci][:, tloc * 4:tloc * 4 + 4]
                    S = (iota[None, :] == m[:, 0:1]) * m[:, 2:3]
                    G = gathered[ci][:, tloc * C2:(tloc + 1) * C2]
                    acc += S.T @ G
            z = acc * dinv[wdx, :, 1:2] + b2[None, :]
            h2[wdx] = np.maximum(z, 0) + a2[None, :] * np.minimum(z, 0)
        lo = h2[:WPH].reshape(-1, C2)[:PCH]
        hi = h2[WPH:].reshape(-1, C2)[:PCH]
        outs.append((lo + hi) * 0.5)
    return np.concatenate(outs, 0)


def _emu_gather(cfg, idx_blocks, calls, table, width):
    out = []
    for ci, (b, ch, tc) in enumerate(calls):
        g = np.zeros((128, max(tc, 1) * width), np.float32)
        if tc:
            cols = tc * 128 // 16
            lay = idx_blocks[ci][:16, :cols]
            idx = lay.T.reshape(-1).astype(np.int64) + ch * cfg["CHUNK_ROWS"]
            rows = table[idx]
            g = rows.reshape(tc, 128, width).transpose(1, 2, 0).transpose(
                0, 2, 1).reshape(128, tc * width)
        out.append(g)
    return out


# ---------------------------------------------------------------------------
# Bass kernel builder
# ---------------------------------------------------------------------------
def build_bass(cfg, structure):
    import os

    import concourse.bass as bass
    import concourse.tile as tile
    from concourse import bacc as bacc_mod
    from concourse import mybir

    stop = os.environ.get("GCN_STOP", "full")  # A | B | C | full

    NWIN, NCHUNK, WB, WPH = cfg["NWIN"], cfg["NCHUNK"], cfg["WB"], cfg["WPH"]
    C1, C2 = cfg["C1"], cfg["C2"]
    SHARD_ROWS, TABLE_ROWS, CHUNK_ROWS = (cfg["SHARD_ROWS"],
                                          cfg["TABLE_ROWS"],
                                          cfg["CHUNK_ROWS"])
    tiles_wc = structure["tiles_wc"]
    calls = structure["calls"]
    mct = structure["max_call_tiles"]
    ncalls = len(calls)
    f32 = mybir.dt.float32
    AF = mybir.ActivationFunctionType
    OP = mybir.AluOpType

    # Bacc (not plain Bass): finalize() runs the TRN2 legalization passes
    # (sync-wait splitting, custom-ISA codegen, library load insertion).
    nc = bacc_mod.Bacc(num_devices=cfg["NCORES"])

    # I/O
    meta_d = nc.declare_dram_parameter("meta", [ncalls * 128, mct * 4], f32,
                                       isOutput=False)
    idx_d = nc.declare_dram_parameter("idx", [ncalls * 128, mct * 8],
                                      mybir.dt.int16, isOutput=False)
    xT_d = nc.declare_dram_parameter("xT", [128, SHARD_ROWS], f32,
                                     isOutput=False)
    W1_d = nc.declare_dram_parameter("W1", [128, C1], f32, isOutput=False)
    W2_d = nc.declare_dram_parameter("W2", [C1, C2], f32, isOutput=False)
    b1_d = nc.declare_dram_parameter("b1r", [128, C1], f32, isOutput=False)
    a1_d = nc.declare_dram_parameter("a1r", [128, C1], f32, isOutput=False)
    b2_d = nc.declare_dram_parameter("b2r", [128, C2], f32, isOutput=False)
    a2_d = nc.declare_dram_parameter("a2r", [128, C2], f32, isOutput=False)
    iota_d = nc.declare_dram_parameter("iota", [128, 128], f32, isOutput=False)
    ident_d = nc.declare_dram_parameter("ident", [128, 128], f32,
                                        isOutput=False)
    out_d = nc.declare_dram_parameter("out", [WPH * 128, C2], f32,
                                      isOutput=True)

    rg = [list(range(cfg["NCORES"]))]

    with tile.TileContext(nc, num_cores=cfg["NCORES"]) as tc_:
        with (
            tc_.tile_pool(name="const", bufs=1) as constp,
            tc_.tile_pool(name="dinv", bufs=1) as dinvp,
            tc_.tile_pool(name="meta", bufs=6) as metap,
            tc_.tile_pool(name="idx", bufs=6) as idxp,
            tc_.tile_pool(name="g", bufs=6) as gp,
            tc_.tile_pool(name="s", bufs=4) as sp,
            tc_.tile_pool(name="ev", bufs=3) as evp,
            tc_.tile_pool(name="stash", bufs=1) as stashp,
            tc_.tile_pool(name="xtp", bufs=3) as xtp,
            tc_.tile_pool(name="degps", bufs=2, space="PSUM") as degps,
            tc_.tile_pool(name="winps", bufs=2, space="PSUM") as winps,
            tc_.tile_pool(name="tps", bufs=2, space="PSUM") as tps,
            tc_.tile_pool(name="y2ps", bufs=2, space="PSUM") as y2ps,
            tc_.tile_pool(name="dram", bufs=1, space="DRAM") as dramp,
        ):
            # ---- constants into SBUF
            iota_sb = constp.tile([128, 128], f32, name="iota_sb")
            ident_sb = constp.tile([128, 128], f32, name="ident_sb")
            W1_sb = constp.tile([128, C1], f32, name="W1_sb")
            W2_sb = constp.tile([C1, C2], f32, name="W2_sb")
            b1_sb = constp.tile([128, C1], f32, name="b1_sb")
            a1_sb = constp.tile([128, C1], f32, name="a1_sb")
            b2_sb = constp.tile([128, C2], f32, name="b2_sb")
            a2_sb = constp.tile([128, C2], f32, name="a2_sb")
            for sb, dr in ((iota_sb, iota_d), (ident_sb, ident_d),
                           (W1_sb, W1_d), (W2_sb, W2_d), (b1_sb, b1_d),
                           (a1_sb, a1_d), (b2_sb, b2_d), (a2_sb, a2_d)):
                nc.sync.dma_start(out=sb, in_=dr[:, :])

            dinv_sb = dinvp.tile([128, NWIN * 2], f32, name="dinv_sb")

            # DRAM scratch
            yw1_shard = dramp.tile([SHARD_ROWS, C1], f32, name="yw1_shard")
            yw1_full = dramp.tile([TABLE_ROWS, C1], f32, name="yw1_full",
                                  addr_space="Shared")
            yw2_shard = dramp.tile([SHARD_ROWS, C2], f32, name="yw2_shard")
            yw2_full = dramp.tile([TABLE_ROWS, C2], f32, name="yw2_full",
                                  addr_space="Shared")

            call_of = {(b, ch): i for i, (b, ch, _) in enumerate(calls)}

            # one Pool register per distinct num_idxs value (to_reg allocates
            # a fresh register per call and the register file is small)
            _nreg_cache = {}

            def nreg(v):
                if v not in _nreg_cache:
                    _nreg_cache[v] = nc.gpsimd.to_reg(v)
                return _nreg_cache[v]

            # ================= pass A: degrees -> dinv =================
            ab = os.environ.get("GCN_AB", "")
            if ab == "nodve":
                s_const = constp.tile([128, 128], f32, name="s_const")
                nc.vector.tensor_copy(out=s_const, in_=iota_sb)
            if ab in ("nomm", "noevict"):
                nc.vector.memset(dinv_sb[:, :], 1.0)
            meta_tiles = {}

            def load_meta(ci):
                t = metap.tile([128, mct * 4], f32, tag="meta")
                nc.sync.dma_start(out=t,
                                  in_=meta_d[ci * 128:(ci + 1) * 128, :])
                return t

            cursor = [0] * ncalls
            for b in range(structure["nbatch"]):
                for ch in range(NCHUNK):
                    ci = call_of[(b, ch)]
                    if calls[ci][2]:
                        meta_tiles[ci] = load_meta(ci)
                wlo = b * WB
                whi = min(wlo + WB, NWIN)
                for wdx in range(wlo, whi):
                    ntile_w = int(tiles_wc[wdx].sum())
                    if ab == "wide":
                        deg_ps = winps.tile([128, 128], f32, tag="win")
                    else:
                        deg_ps = degps.tile([128, 2], f32, tag="deg")
                    k = 0
                    for ch in range(NCHUNK):
                        ci = call_of[(b, ch)]
                        for _ in range(int(tiles_wc[wdx, ch])):
                            tloc = cursor[ci]
                            cursor[ci] += 1
                            m = meta_tiles[ci]
                            if ab == "nodve":
                                s_t = s_const
                            else:
                                s_t = sp.tile([128, 128], f32, tag="s")
                                nc.vector.tensor_scalar(
                                    out=s_t, in0=iota_sb,
                                    scalar1=(5.0 if ab == "imm" else
                                             m[:, 4 * tloc:4 * tloc + 1]),
                                    scalar2=None, op0=OP.is_equal)
                            if ab == "nomm":
                                pass
                            elif ab == "wide":
                                nc.tensor.matmul(
                                    out=deg_ps, lhsT=s_t, rhs=iota_sb,
                                    start=(k == 0), stop=(k == ntile_w - 1))
                            else:
                                nc.tensor.matmul(
                                    out=deg_ps, lhsT=s_t,
                                    rhs=m[:, 4 * tloc + 1:4 * tloc + 3],
                                    start=(k == 0), stop=(k == ntile_w - 1))
                            k += 1
                    # dinv = 1/sqrt(deg); deg >= 1 (self loop).  Rsqrt is
                    # banned in bass (accuracy) -> reciprocal then sqrt.
                    if ab in ("nomm", "noevict"):
                        continue
                    rec_t = evp.tile([128, 2], f32, tag="rec")
                    nc.vector.reciprocal(out=rec_t, in_=deg_ps[:, 0:2])
                    nc.scalar.activation(
                        out=dinv_sb[:, 2 * wdx:2 * wdx + 2], in_=rec_t,
                        func=AF.Sqrt)

            # ================= pass B: yw1 shard + AllGather ============
            if stop == "A":
                nc.sync.dma_start(out=out_d[0:128, :],
                                  in_=dinv_sb[:, 0:C2])
            if stop in ("B", "C0", "C1", "C", "full"):
                for wdx in range(NWIN):
                    xt_t = xtp.tile([128, 128], f32, tag="xt")
                    nc.sync.dma_start(out=xt_t,
                                      in_=xT_d[:, wdx * 128:(wdx + 1) * 128])
                    xw_ps = y2ps.tile([128, C1], f32, tag="y2")
                    nc.tensor.matmul(out=xw_ps, lhsT=xt_t, rhs=W1_sb,
                                     start=True, stop=True)
                    yw_t = evp.tile([128, C1], f32, tag="yw")
                    nc.vector.tensor_scalar(
                        out=yw_t, in0=xw_ps,
                        scalar1=dinv_sb[:, 2 * wdx:2 * wdx + 1],
                        scalar2=None, op0=OP.mult)
                    nc.sync.dma_start(
                        out=yw1_shard[wdx * 128:(wdx + 1) * 128, :], in_=yw_t)

                nc.gpsimd.collective_compute(
                    "AllGather", OP.bypass, replica_groups=rg,
                    ins=[yw1_shard[:, :]], outs=[yw1_full[:, :]])
            if stop == "B":
                t_dbg = evp.tile([128, C2], f32, tag="dbg")
                nc.sync.dma_start(out=t_dbg, in_=yw1_full[0:128, 0:C2])
                nc.sync.dma_start(out=out_d[0:128, :], in_=t_dbg)

            # ============ pass C: layer-1 messages, fused yw2 ===========
            def msg_pass(table, width, wcol, dcol, b_sb, a_sb, out_cb):
                """wcol: meta column for edge weight (1=w, 2=type);
                dcol: dinv column (0 or 1) used at eviction;
                out_cb(wdx, h_tile): consume the [128, width] result."""
                cursor = [0] * ncalls
                for b in range(structure["nbatch"]):
                    g_tiles = {}
                    for ch in range(NCHUNK):
                        ci = call_of[(b, ch)]
                        tcn = calls[ci][2]
                        if not tcn:
                            continue
                        meta_tiles[ci] = load_meta(ci)
                        it = idxp.tile([128, mct * 8], mybir.dt.int16,
                                       tag="idx")
                        nc.sync.dma_start(
                            out=it, in_=idx_d[ci * 128:(ci + 1) * 128, :])
                        g_t = gp.tile([128, mct * C1], f32, tag="g")
                        if not os.environ.get("GCN_NOGATHER"):
                            nc.gpsimd.dma_gather(
                                out_ap=g_t[:, :tcn * width].rearrange(
                                    "p (t e) -> p t e", e=width),
                                in_ap=table[ch * CHUNK_ROWS:
                                            (ch + 1) * CHUNK_ROWS, :],
                                idxs_ap=it[:, :tcn * 8],
                                num_idxs=tcn * 128,
                                num_idxs_reg=nreg(tcn * 128),
                                elem_size=width,
                                # single_packet=True breaks for calls over
                                # ~384 indices (HW-bisected)
                                single_packet=False)
                        else:
                            nc.vector.tensor_copy(out=g_t[:, 0:128],
                                                  in_=iota_sb)
                        g_tiles[ch] = g_t
                    wlo = b * WB
                    whi = min(wlo + WB, NWIN)
                    for wdx in range(wlo, whi):
                        ntile_w = int(tiles_wc[wdx].sum())
                        h_ps = winps.tile([128, width], f32, tag="win")
                        k = 0
                        for ch in range(NCHUNK):
                            ci = call_of[(b, ch)]
                            for _ in range(int(tiles_wc[wdx, ch])):
                                tloc = cursor[ci]
                                cursor[ci] += 1
                                m = meta_tiles[ci]
                                s_t = sp.tile([128, 128], f32, tag="s")
                                nc.vector.tensor_scalar(
                                    out=s_t, in0=iota_sb,
                                    scalar1=m[:, 4 * tloc:4 * tloc + 1],
                                    scalar2=m[:, 4 * tloc + wcol:
                                              4 * tloc + wcol + 1],
                                    op0=OP.is_equal, op1=OP.mult)
                                nc.tensor.matmul(
                                    out=h_ps, lhsT=s_t,
                                    rhs=g_tiles[ch][:, tloc * width:
                                                    (tloc + 1) * width],
                                    start=(k == 0), stop=(k == ntile_w - 1))
                                k += 1
                        # evict: z = psum * dinv + b ; h = prelu(z, a)
                        if os.environ.get("GCN_NOEVICT"):
                            h_t = evp.tile([128, width], f32, tag="h")
                            nc.vector.tensor_copy(out=h_t, in_=h_ps)
                            out_cb(wdx, h_t)
                            continue
                        dv = dinv_sb[:, 2 * wdx + dcol:2 * wdx + dcol + 1]
                        z_t = evp.tile([128, width], f32, tag="z")
                        nc.vector.scalar_tensor_tensor(
                            out=z_t, in0=h_ps, scalar=dv, in1=b_sb,
                            op0=OP.mult, op1=OP.add)
                        mn_t = evp.tile([128, width], f32, tag="mn")
                        nc.vector.tensor_scalar(
                            out=mn_t, in0=z_t, scalar1=0.0, scalar2=None,
                            op0=OP.min)
                        am_t = evp.tile([128, width], f32, tag="am")
                        nc.vector.tensor_tensor(out=am_t, in0=mn_t, in1=a_sb,
                                                op=OP.mult)
                        h_t = evp.tile([128, width], f32, tag="h")
                        nc.vector.scalar_tensor_tensor(
                            out=h_t, in0=z_t, scalar=0.0, in1=am_t,
                            op0=OP.max, op1=OP.add)
                        out_cb(wdx, h_t)

            def l1_out(wdx, h_t):
                # fused layer-2 dense: yw2 = (h1 @ W2) * dinv2
                t_ps = tps.tile([128, 128], f32, tag="tp")
                nc.tensor.transpose(out=t_ps, in_=h_t, identity=ident_sb)
                h1T = evp.tile([128, 128], f32, tag="h1T")
                nc.vector.tensor_copy(out=h1T, in_=t_ps)
                y2_ps = y2ps.tile([128, C2], f32, tag="y2")
                nc.tensor.matmul(out=y2_ps, lhsT=h1T, rhs=W2_sb,
                                 start=True, stop=True)
                yw2_t = evp.tile([128, C2], f32, tag="yw2")
                nc.vector.tensor_scalar(
                    out=yw2_t, in0=y2_ps,
                    scalar1=dinv_sb[:, 2 * wdx + 1:2 * wdx + 2],
                    scalar2=None, op0=OP.mult)
                nc.sync.dma_start(
                    out=yw2_shard[wdx * 128:(wdx + 1) * 128, :], in_=yw2_t)

            def l1_out_nofuse(wdx, h_t):
                nc.sync.dma_start(
                    out=yw2_shard[wdx * 128:(wdx + 1) * 128, :],
                    in_=h_t[:, :C2])

            if stop in ("C0",):
                msg_pass(yw1_full, C1, 1, 0, b1_sb, a1_sb, l1_out_nofuse)
            if stop in ("C1",):
                msg_pass(yw1_full, C1, 1, 0, b1_sb, a1_sb, l1_out)
            if stop in ("C", "full"):
                msg_pass(yw1_full, C1, 1, 0, b1_sb, a1_sb, l1_out)

                nc.gpsimd.collective_compute(
                    "AllGather", OP.bypass, replica_groups=rg,
                    ins=[yw2_shard[:, :]], outs=[yw2_full[:, :]])
            if stop == "C":
                t_dbg = evp.tile([128, C2], f32, tag="dbg")
                nc.sync.dma_start(out=t_dbg, in_=yw2_full[0:128, :])
                nc.sync.dma_start(out=out_d[0:128, :], in_=t_dbg)
            if stop in ("C0", "C1"):
                t_dbg = evp.tile([128, C2], f32, tag="dbg")
                nc.sync.dma_start(out=t_dbg, in_=yw2_shard[0:128, :])
                nc.sync.dma_start(out=out_d[0:128, :], in_=t_dbg)

            # ============ pass E: layer-2 messages + combine ============
            stash = stashp.tile([128, WPH * C2], f32, name="h2lo")

            def l2_out(wdx, h_t):
                if wdx < WPH:
                    nc.vector.tensor_copy(
                        out=stash[:, wdx * C2:(wdx + 1) * C2], in_=h_t)
                else:
                    w2 = wdx - WPH
                    cmb = evp.tile([128, C2], f32, tag="cmb")
                    nc.vector.tensor_tensor(
                        out=cmb, in0=h_t,
                        in1=stash[:, w2 * C2:(w2 + 1) * C2], op=OP.add)
                    o_t = evp.tile([128, C2], f32, tag="o")
                    nc.vector.tensor_scalar(
                        out=o_t, in0=cmb, scalar1=0.5, scalar2=None,
                        op0=OP.mult)
                    nc.sync.dma_start(
                        out=out_d[w2 * 128:(w2 + 1) * 128, :], in_=o_t)

            if stop == "full":
                msg_pass(yw2_full, C2, 2, 1, b2_sb, a2_sb, l2_out)

    nc.finalize()
    return nc


# ---------------------------------------------------------------------------
# Host driver
# ---------------------------------------------------------------------------
def make_in_maps(cfg, structure, per_core, W1, b1, a1, W2, b2, a2):
    mct = structure["max_call_tiles"]
    ncalls = len(structure["calls"])
    iota = np.tile(np.arange(128, dtype=np.float32), (128, 1))
    ident = np.eye(128, dtype=np.float32)
    consts = dict(
        W1=np.ascontiguousarray(W1, np.float32),
        W2=np.ascontiguousarray(W2, np.float32),
        b1r=np.tile(b1.astype(np.float32), (128, 1)),
        a1r=np.tile(a1.astype(np.float32), (128, 1)),
        b2r=np.tile(b2.astype(np.float32), (128, 1)),
        a2r=np.tile(a2.astype(np.float32), (128, 1)),
        iota=np.ascontiguousarray(iota),
        ident=ident,
    )
    in_maps = []
    for c in range(cfg["NCORES"]):
        pc = per_core[c]
        in_maps.append(dict(
            meta=pc["meta"].reshape(ncalls * 128, mct * 4),
            idx=pc["idx"].reshape(ncalls * 128, mct * 8),
            xT=pc["xT"],
            **consts,
        ))
    return in_maps


def assemble_out(cfg, outs):
    """outs: list per core of the 'out' array [WPH*128, C2]."""
    parts = [o[:cfg["PCH"]] for o in outs]
    return np.ascontiguousarray(np.concatenate(parts, 0), dtype=np.float32)


LAST_EXEC_NS = None


def _trivial_nc(ncores):
    """A minimal bass kernel for dispatch-overhead calibration."""
    from concourse import bacc as bacc_mod
    from concourse import mybir
    import concourse.tile as tile

    f32 = mybir.dt.float32
    nc = bacc_mod.Bacc(num_devices=ncores)
    a = nc.declare_dram_parameter("a", [128, 128], f32, isOutput=False)
    o = nc.declare_dram_parameter("o", [128, 128], f32, isOutput=True)
    with tile.TileContext(nc, num_cores=ncores) as tc:
        with tc.tile_pool(name="p", bufs=2) as p:
            t = p.tile([128, 128], f32)
            nc.sync.dma_start(out=t, in_=a[:, :])
            nc.sync.dma_start(out=o[:, :], in_=t)
    nc.finalize()
    return nc


def _time_kernel(nc, in_maps, n_cores, iters=5, reps=10, n_iters=None):
    """Best-of-N wall time of one dispatched execution (device put inputs,
    block_until_ready).  Subtract a trivial-kernel baseline for HW time.
    If n_iters is given, run the kernel n_iters times inside one dispatch
    (plus a 1-iter program) and return (T(n) - T(1)) / (n - 1), which
    cancels dispatch overhead far more precisely."""
    import time

    import jax
    import numpy as np
    from jax.experimental.shard_map import shard_map
    from jax.sharding import Mesh, PartitionSpec

    from concourse import bass2jax, mybir

    bass2jax.install_neuronx_cc_hook()
    partition_name = (nc.partition_id_tensor.name
                      if nc.partition_id_tensor else None)
    in_names, out_names, out_avals, zero_outs = [], [], [], []
    for alloc in nc.m.functions[0].allocations:
        if not isinstance(alloc, mybir.MemoryLocationSet):
            continue
        name = alloc.memorylocations[0].name
        if alloc.kind == "ExternalInput":
            if name != partition_name:
                in_names.append(name)
        elif alloc.kind == "ExternalOutput":
            out_names.append(name)
            shape = tuple(alloc.tensor_shape)
            dtype = mybir.dt.np(alloc.dtype)
            out_avals.append(jax.core.ShapedArray(shape, dtype))
            zero_outs.append(np.zeros(shape, dtype))
    n_params = len(in_names)
    all_in_names = list(in_names) + list(out_names)
    if partition_name is not None:
        all_in_names.append(partition_name)

    n_outs_ = len(out_names)

    def make_body(n):
        # each iteration gets its own zero-buffer parameters: operands stay
        # top-level parameters (hook requirement) and differ across
        # iterations (no CSE); the effectful primitive keeps them ordered.
        def _body(*args):
            ins = list(args[:n_params])
            outs = None
            for i in range(n):
                zs = list(args[n_params + i * n_outs_:
                               n_params + (i + 1) * n_outs_])
                operands = ins + zs
                if partition_name is not None:
                    operands.append(bass2jax.partition_id_tensor())
                outs = bass2jax._bass_exec_p.bind(
                    *operands,
                    out_avals=tuple(out_avals),
                    in_names=tuple(all_in_names),
                    out_names=tuple(out_names),
                    lowering_input_output_aliases=(),
                    sim_require_finite=False,
                    sim_require_nnan=False,
                    nc=nc,
                )
            return tuple(outs)
        return _body

    devices = jax.devices()[:n_cores]
    mesh = Mesh(np.asarray(devices), ("core",))

    per_core = [[np.asarray(m[name]) for name in in_names] for m in in_maps]
    concat_in = [np.concatenate([per_core[c][i] for c in range(n_cores)], 0)
                 for i in range(n_params)]
    concat_zeros = [np.zeros((n_cores * z.shape[0], *z.shape[1:]), z.dtype)
                    for z in zero_outs]
    dev_in = [jax.device_put(a) for a in concat_in]
    dev_zero = [jax.device_put(a) for a in concat_zeros]

    n = 1
    in_specs = (PartitionSpec("core"),) * (n_params + n * n_outs_)
    out_specs = (PartitionSpec("core"),) * n_outs_
    fn = jax.jit(shard_map(make_body(n), mesh=mesh, in_specs=in_specs,
                           out_specs=out_specs, check_rep=False),
                 keep_unused=True)
    args = dev_in + dev_zero * n
    out = fn(*args)  # compile + warm
    jax.block_until_ready(out)

    def one(k):
        t0 = time.perf_counter()
        out = None
        for _ in range(k):
            out = fn(*args)
        jax.block_until_ready(out)
        return time.perf_counter() - t0

    if n_iters is not None and n_iters > 1:
        # interleave k=n_iters and k=1 reps so drift affects both equally;
        # slope of the best pair isolates per-dispatch device time.
        k = n_iters
        his, los = [], []
        one(1)
        for _ in range(reps):
            his.append(one(k))
            los.append(one(1))
        his.sort()
        los.sort()
        print(f"  timing(k={k}): {[f'{t*1e3:.1f}' for t in his[:6]]}")
        print(f"  timing(k=1): {[f'{t*1e3:.1f}' for t in los[:6]]}")
        per = (his[0] - los[0]) / (k - 1) * 1e9
        print(f"  timing: slope {per/1e6:.3f} ms/dispatch")
        return per
    return one(1) * 1e9


def kernel(**inputs):
    global LAST_EXEC_NS
    import os

    cfg = CFG
    x = np.asarray(inputs["x"], np.float32)
    W1 = np.asarray(inputs["W1"], np.float32)
    b1 = np.asarray(inputs["b1"], np.float32)
    a1 = np.asarray(inputs["a1"], np.float32)
    W2 = np.asarray(inputs["W2"], np.float32)
    b2 = np.asarray(inputs["b2"], np.float32)
    a2 = np.asarray(inputs["a2"], np.float32)

    structure, per_core = prep(cfg, x, inputs["edge_index"],
                               inputs["edge_weight"], inputs["edge_type"])

    if os.environ.get("GCN_EMULATE"):
        return emulate(cfg, structure, per_core, W1, b1, a1, W2, b2, a2)

    from concourse.bass_utils import run_bass_kernel_spmd

    nc = build_bass(cfg, structure)
    in_maps = make_in_maps(cfg, structure, per_core, W1, b1, a1, W2, b2, a2)
    res = run_bass_kernel_spmd(
        nc, in_maps, core_ids=list(range(cfg["NCORES"])))
    LAST_EXEC_NS = res.exec_time_ns
    if os.environ.get("GCN_TIME"):
        n_it = int(os.environ.get("GCN_ITERS", "9"))
        main_ns = _time_kernel(nc, in_maps, cfg["NCORES"], n_iters=n_it)
        LAST_EXEC_NS = main_ns
    return assemble_out(cfg, [res.results[c]["out"]
                              for c in range(cfg["NCORES"])])

